# revision 7
# baseline (speedup 1.0000x reference)
"""ColBERT MaxSim kernel for 8 Trainium2 NeuronCores.

scores[b, c] = sum_n max_s (qs[b, n, :] . ps[c, s, :])
  qs: (64, 32, 128) f32, ps: (64, 1024, 128) f32 -> scores: (64, 64) f32

Sharding: docs (c) are sharded 8 per core; qs is replicated. Each core
computes its (64, 8) score tile; the host concatenates along c.

Mode "fast" (default) per-core dataflow:
  - Doc tokens are combined in PAIRS on the host: P+ = (Pe+Po)/2,
    P- = (Pe-Po)/2, so max(a,b) = S + |D| with S = Q.P+, D = Q.P-.
  - The kernel is PSUM-drain-bound: every sim-derivative must cross
    PSUM->engine at 1 elem/lane/cycle, and only ScalarE (1.2 GHz) and
    VectorE (0.96 GHz) can read PSUM (DMA/GpSimd have no PSUM route).
    Per (M-group, doc) tile the drain is 512 (D, via ScalarE Abs) +
    512 (S, via VectorE reduce_max) - an even, optimal 2-engine split.
  - Docs are processed in PAIRS: one ACTIVATE Abs over the two D banks
    [128, 1024] (~1110 ns, vs 2x690 unbatched) and one reduce_max over
    [128, 2, 512] (~1224 ns). The steady-state period is then bound by
    the PE at 6 x 216 ns = ~1295 ns per 2-doc group (4 data matmuls +
    2 identity folds), with ScalarE/VectorE just under it.
  - The D matmuls are emitted under tc.high_priority(offset=8): the
    Tile scheduler otherwise hoists the S matmuls (which wait on the
    2-generations-back reduce) ahead of them, starving the Abs and
    inflating the period ~20%. Offset 8 gives exactly one group of
    D-lookahead (16 let two groups jump the first identity folds,
    stretching pipeline fill by ~1.7us).
  - fp16 everywhere: fp8 operands trigger a ~1.2x whole-core clock
    throttle (measured: every engine slows 20% when fp8 matmuls are in
    the stream), and fp8 DoubleRow additionally loses FWL. fp16 rel
    err vs the f32 reference: ~3e-5 (tolerance 2e-2).
  - Head: ~7.5 us NEFF preamble (fixed), then input DMA in per-doc
    first chunks split across both HWDGE queues; HAM warmup matmuls
    (PE needs ~5 us of sustained activity to lift the clock gate
    1.2 -> 2.4 GHz) bridge the preamble->data window.
  - Tail: token-sums run as float32r (single-pass) matmuls; docs 0-5
    flush mid-stream, so only the last 2 columns' fin+copy+DMA sit
    after the last reduce. A keep-alive DMA gated on the 2nd-to-last
    group's maxcols keeps the sync queue mid-stream, cutting the final
    transfer's completion latency 2.1 -> 1.5 us.

Mode "pair" is the previous all-fp16 per-doc-abs kernel, mode
"direct" the exact-fp32 fallback.
"""

import os
import sys
from contextlib import ExitStack

import numpy as np

sys.path.insert(0, "/opt/trn_rl_repo")
sys.path.insert(0, "/opt/trn_rl_repo/concourse")

import bass_rust
import concourse.bass as bass
import concourse.mybir as mybir
import concourse.tile as tile
from concourse import bass_utils

# Problem shape (hardcoded per contract)
N_CORES = 8
NQ, TQ, D = 64, 32, 128          # queries, query tokens, dim
ND, TD = 64, 1024                # docs, doc tokens
DOCS_PER_CORE = ND // N_CORES    # 8
QROWS = NQ * TQ                  # 2048 query-token rows
MG = QROWS // 128                # 16 M-groups of 128 rows
QPG = 128 // TQ                  # 4 queries per M-group
NPAIR = TD // 2                  # 512 token pairs per doc

F32 = mybir.dt.float32
F32R = mybir.dt.float32r
F16 = mybir.dt.float16
FP8 = mybir.dt.float8e4

MODE = os.environ.get("KERNEL_MODE", "fast")
FDAT = FP8 if os.environ.get("KERNEL_FP8", "0") == "1" else F16


def _split_multi_waits(nc):
    """This walrus build rejects >1 embedded sync wait per instruction
    ("Too many sync wait commands"). Split extras onto single-wait NoOps
    inserted just before the instruction on the same engine — semantically
    identical (per-engine program order is preserved)."""
    n_split = 0
    for fn in nc.m.functions:
        for blk in fn.blocks:
            out = []
            for ins in blk.instructions:
                si = ins.sync_info
                waits = list(si.on_wait) if si and si.on_wait else []
                if len(waits) > 1:
                    for j, w in enumerate(waits[:-1]):
                        nop = mybir.InstNoOp(
                            name=f"{ins.name}_sw{j}", ins=[], outs=[])
                        nop.engine = ins.engine
                        nop.sync_info = bass_rust.SyncInfo(
                            on_wait=[w], on_update=[])
                        out.append(nop)
                    ins.sync_info = bass_rust.SyncInfo(
                        on_wait=[waits[-1]], on_update=list(si.on_update))
                    n_split += 1
                out.append(ins)
            blk.instructions = out
    return n_split


def _build_fast_module(split_first=True):
    # split_first: emit the group-0 matmuls as two half-bank matmuls so
    # they gate on the half-chunk DMAs. Semantically identical; CoreSim's
    # per-bank PSUM zero-tracking rejects it, so sim builds disable it.
    nc = bass.Bass("TRN2", target_bir_lowering=False, debug=False)

    qsT = nc.dram_tensor("qsT", [D, QROWS], FDAT, kind="ExternalInput").ap()
    psP = nc.dram_tensor("psP", [D, DOCS_PER_CORE * NPAIR], FDAT,
                         kind="ExternalInput").ap()
    psM = nc.dram_tensor("psM", [D, DOCS_PER_CORE * NPAIR], FDAT,
                         kind="ExternalInput").ap()
    ident = nc.dram_tensor("ident", [128, 128], F16,
                           kind="ExternalInput").ap()
    ones = nc.dram_tensor("ones", [128, QPG], F32R,
                          kind="ExternalInput").ap()
    out = nc.dram_tensor("out", [NQ, DOCS_PER_CORE], F32,
                         kind="ExternalOutput").ap()

    with tile.TileContext(nc) as tc, ExitStack() as ctx:
        const = ctx.enter_context(tc.tile_pool(name="const", bufs=1))
        stage = ctx.enter_context(tc.tile_pool(name="stage", bufs=4))
        psumS = ctx.enter_context(
            tc.tile_pool(name="psumS", bufs=2, space="PSUM"))
        psumD = ctx.enter_context(
            tc.tile_pool(name="psumD", bufs=2, space="PSUM"))

        qsT_sb = const.tile([D, QROWS], FDAT)
        psP_sb = const.tile([D, DOCS_PER_CORE * NPAIR], FDAT)
        psM_sb = const.tile([D, DOCS_PER_CORE * NPAIR], FDAT)
        ident_sb = const.tile([128, 128], F16)
        ones_sb = const.tile([128, QPG], F32R)
        scratch = const.tile([128, QPG], F32R)

        # First chunks cover doc 0 (in halves, so the group-0 matmuls can
        # gate on quarter-transfers and dodge straggler DMA engine-streams
        # — measured: 1 of 16 streams can land ~1.7us after the other 15),
        # then doc 1, then the rest. Issues spread across THREE HWDGE
        # queues (sync + scalar + vector) to parallelize the ~650ns
        # descriptor writes and the queue spin-up.
        q0 = 256        # M-groups 0-1
        H = NPAIR // 2
        nc.sync.dma_start(qsT_sb[:, 0:q0], qsT[:, 0:q0])
        nc.scalar.dma_start(psM_sb[:, 0:H], psM[:, 0:H])
        nc.sync.dma_start(psP_sb[:, 0:H], psP[:, 0:H])
        nc.scalar.dma_start(psM_sb[:, H:NPAIR], psM[:, H:NPAIR])
        nc.sync.dma_start(psP_sb[:, H:NPAIR], psP[:, H:NPAIR])
        nc.scalar.dma_start(psP_sb[:, NPAIR:2 * NPAIR],
                            psP[:, NPAIR:2 * NPAIR])
        nc.sync.dma_start(psM_sb[:, NPAIR:2 * NPAIR],
                          psM[:, NPAIR:2 * NPAIR])
        # Prefetch the Abs ACT table set (~2.7us TABLE_LOAD + drain) NOW so
        # it overlaps the initial DMA instead of gating the first real abs.
        warm = stage.tile([1, 2], F16, tag="warm")
        nc.gpsimd.memset(warm[:], 0.0)
        warm2 = stage.tile([1, 2], F16, tag="warm2")
        nc.scalar.activation(warm2[:], warm[:],
                             mybir.ActivationFunctionType.Abs)
        nc.scalar.dma_start(ident_sb[:], ident[:])
        nc.sync.dma_start(qsT_sb[:, q0:1152], qsT[:, q0:1152])
        nc.scalar.dma_start(qsT_sb[:, 1152:], qsT[:, 1152:])
        nc.sync.dma_start(ones_sb[:], ones[:])
        # Bulk for docs 2-7 (first consumed ~20us in) rides the gpsimd
        # SWDGE queue — frees the two HWDGE queues for the urgent chunks
        # and parallelizes descriptor issue.
        nc.gpsimd.dma_start(psM_sb[:, 2 * NPAIR:5 * NPAIR],
                            psM[:, 2 * NPAIR:5 * NPAIR])
        nc.gpsimd.dma_start(psP_sb[:, 2 * NPAIR:5 * NPAIR],
                            psP[:, 2 * NPAIR:5 * NPAIR])
        nc.gpsimd.dma_start(psM_sb[:, 5 * NPAIR:], psM[:, 5 * NPAIR:])
        nc.gpsimd.dma_start(psP_sb[:, 5 * NPAIR:], psP[:, 5 * NPAIR:])

        # HAM warmup: the PE needs ~5us of sustained activity to lift the
        # clock gate from 1.2 to 2.4 GHz; these matmuls bridge the NEFF
        # preamble -> first-DMA-chunk window so the real stream starts as
        # early and as warm as possible.
        garbage = const.tile([128, NPAIR], F16)
        nc.gpsimd.memset(garbage[:], 0.0)
        for _ in range(4):
            wt = psumD.tile([128, 2 * NPAIR], F32, tag="d")
            nc.tensor.matmul(wt[:, 0:384], lhsT=garbage[:, 0:128],
                             rhs=garbage[:, 0:384], start=True, stop=True)
        for _ in range(2):
            wt = psumD.tile([128, 2 * NPAIR], F32, tag="d")
            nc.tensor.matmul(wt[:, 0:384], lhsT=qsT_sb[:, 0:128],
                             rhs=garbage[:, 0:384], start=True, stop=True)

        # maxcols[p, mg*8 + dloc] = max over doc dloc's tokens for row p
        # of mg; float32r so the single-pass fin matmuls may consume it.
        maxcols = const.tile([128, MG * DOCS_PER_CORE], F32R)
        out_sb = const.tile([QPG, MG * DOCS_PER_CORE], F32)

        def emit_fin(d0, d1, m0=0, m1=MG):
            # Token-sum + copy + out-DMA for doc columns [d0, d1) of
            # M-groups [m0, m1) (float32r = single-pass matmul). Chunks
            # whose reduces are already done run mid-stream (fin borrows
            # a "d" PSUM slot between Abs consumers); only a [4, 2]
            # micro-chunk for the last group sits on the critical tail,
            # and each chunk's DMA keeps the sync queue warm for the next.
            nd = d1 - d0
            nm = m1 - m0
            mc3 = maxcols[:].rearrange("p (mg d) -> p mg d",
                                       d=DOCS_PER_CORE)
            fin = psumD.tile([QPG, nm * nd], F32, tag="d")
            nc.tensor.matmul(fin[:].rearrange("q (mg d) -> q mg d", d=nd),
                             lhsT=ones_sb[:],
                             rhs=mc3[:, m0:m1, d0:d1],
                             start=True, stop=True)
            oc = out_sb[:].rearrange("q (mg d) -> q mg d",
                                     d=DOCS_PER_CORE)[:, m0:m1, d0:d1]
            nc.vector.tensor_copy(
                oc, fin[:].rearrange("q (mg d) -> q mg d", d=nd))
            out_r = out.rearrange("(mg q) d -> q mg d", q=QPG)
            nc.sync.dma_start(
                out_r[:, m0:m1, d0:d1],
                out_sb[:].rearrange("q (mg d) -> q mg d",
                                    d=DOCS_PER_CORE)[:, m0:m1, d0:d1])

        for dp in range(DOCS_PER_CORE // 2):
            for mg in range(MG):
                if dp == 3 and mg == 6:
                    emit_fin(0, 6)
                if dp == 3 and mg == 13:
                    # keep the sync DMA queue awake for the final transfer
                    nc.sync.dma_start(scratch[:], ones[:])
                lhsT = qsT_sb[:, mg * 128:(mg + 1) * 128]
                d2 = psumD.tile([128, 2 * NPAIR], F32, tag="d")
                s2 = psumS.tile([128, 2 * NPAIR], F32, tag="s")
                # D matmuls first, at high priority: the batched Abs can
                # start as soon as both land, and never sits behind S
                # matmuls stalled on the previous reduce.
                first = split_first and dp == 0 and mg == 0
                with tc.high_priority(offset=8):
                    for h in range(2):
                        dloc = 2 * dp + h
                        sl = slice(dloc * NPAIR, (dloc + 1) * NPAIR)
                        if first and h == 0:
                            # Gate on the two half-chunk DMAs separately so
                            # a straggler stream only delays half the work.
                            nc.tensor.matmul(d2[:, 0:H], lhsT=lhsT,
                                             rhs=psM_sb[:, 0:H],
                                             start=True, stop=True,
                                             skip_group_check=True)
                            nc.tensor.matmul(d2[:, H:NPAIR], lhsT=lhsT,
                                             rhs=psM_sb[:, H:NPAIR],
                                             start=True, stop=True,
                                             skip_group_check=True)
                            continue
                        nc.tensor.matmul(d2[:, h * NPAIR:(h + 1) * NPAIR],
                                         lhsT=lhsT, rhs=psM_sb[:, sl],
                                         start=True, stop=True,
                                         skip_group_check=True)
                for h in range(2):
                    dloc = 2 * dp + h
                    sl = slice(dloc * NPAIR, (dloc + 1) * NPAIR)
                    if first and h == 0:
                        nc.tensor.matmul(s2[:, 0:H], lhsT=lhsT,
                                         rhs=psP_sb[:, 0:H],
                                         start=True, stop=False,
                                         skip_group_check=True)
                        nc.tensor.matmul(s2[:, H:NPAIR], lhsT=lhsT,
                                         rhs=psP_sb[:, H:NPAIR],
                                         start=True, stop=False,
                                         skip_group_check=True)
                        continue
                    nc.tensor.matmul(s2[:, h * NPAIR:(h + 1) * NPAIR],
                                     lhsT=lhsT, rhs=psP_sb[:, sl],
                                     start=True, stop=False,
                                     skip_group_check=True)
                # One batched Abs over both docs' D banks: fewer ACTIVATE
                # fixed costs (352 cyc each) and half the sem traffic.
                a = stage.tile([128, 2 * NPAIR], F16)
                nc.scalar.activation(a[:], d2[:],
                                     mybir.ActivationFunctionType.Abs)
                for h in range(2):
                    nc.tensor.matmul(s2[:, h * NPAIR:(h + 1) * NPAIR],
                                     lhsT=ident_sb[:],
                                     rhs=a[:, h * NPAIR:(h + 1) * NPAIR],
                                     start=False, stop=True,
                                     skip_group_check=True)
                col = mg * DOCS_PER_CORE + 2 * dp
                nc.vector.reduce_max(
                    maxcols[:, col:col + 2],
                    s2[:].rearrange("p (h n) -> p h n", h=2),
                    axis=mybir.AxisListType.X)

        # Late keep-alive: reads the 2nd-to-last group's maxcols columns,
        # so it issues ~1.3us before the final out-DMA and the sync queue
        # is mid-stream (not cold) when the last transfer arrives.
        nc.sync.dma_start(scratch[:, 0:2], maxcols[:, 118:120])
        emit_fin(6, 8)

    return nc


def _build_pair_module():
    nc = bass.Bass("TRN2", target_bir_lowering=False, debug=False)

    qsT = nc.dram_tensor("qsT", [D, QROWS], F16, kind="ExternalInput").ap()
    psP = nc.dram_tensor("psP", [D, DOCS_PER_CORE * NPAIR], F16,
                         kind="ExternalInput").ap()
    psM = nc.dram_tensor("psM", [D, DOCS_PER_CORE * NPAIR], F16,
                         kind="ExternalInput").ap()
    ident = nc.dram_tensor("ident", [128, 128], F16,
                           kind="ExternalInput").ap()
    ones = nc.dram_tensor("ones", [128, QPG], F32, kind="ExternalInput").ap()
    out = nc.dram_tensor("out", [NQ, DOCS_PER_CORE], F32,
                         kind="ExternalOutput").ap()

    with tile.TileContext(nc) as tc, ExitStack() as ctx:
        const = ctx.enter_context(tc.tile_pool(name="const", bufs=1))
        stage = ctx.enter_context(tc.tile_pool(name="stage", bufs=10))
        psumS = ctx.enter_context(
            tc.tile_pool(name="psumS", bufs=2, space="PSUM"))
        psumD = ctx.enter_context(
            tc.tile_pool(name="psumD", bufs=4, space="PSUM"))

        qsT_sb = const.tile([D, QROWS], F16)
        psP_sb = const.tile([D, DOCS_PER_CORE * NPAIR], F16)
        psM_sb = const.tile([D, DOCS_PER_CORE * NPAIR], F16)
        ident_sb = const.tile([128, 128], F16)
        ones_sb = const.tile([128, QPG], F32)
        c0 = 2 * NPAIR
        q0 = 256
        nc.sync.dma_start(qsT_sb[:, 0:q0], qsT[:, 0:q0])
        nc.scalar.dma_start(psM_sb[:, 0:c0], psM[:, 0:c0])
        nc.sync.dma_start(psP_sb[:, 0:c0], psP[:, 0:c0])
        warm = stage.tile([1, 2], F16, tag="warm")
        nc.gpsimd.memset(warm[:], 0.0)
        warm2 = stage.tile([1, 2], F16, tag="warm2")
        nc.scalar.activation(warm2[:], warm[:],
                             mybir.ActivationFunctionType.Abs)
        nc.scalar.dma_start(ident_sb[:], ident[:])
        nc.sync.dma_start(qsT_sb[:, q0:], qsT[:, q0:])
        nc.scalar.dma_start(psM_sb[:, c0:], psM[:, c0:])
        nc.sync.dma_start(psP_sb[:, c0:], psP[:, c0:])
        nc.sync.dma_start(ones_sb[:], ones[:])

        garbage = const.tile([128, NPAIR], F16)
        nc.gpsimd.memset(garbage[:], 0.0)
        for _ in range(12):
            wt = psumD.tile([128, NPAIR], F32, tag="d")
            nc.tensor.matmul(wt[:], lhsT=garbage[:, 0:128], rhs=garbage[:],
                             start=True, stop=True)
        for _ in range(6):
            wt = psumD.tile([128, NPAIR], F32, tag="d")
            nc.tensor.matmul(wt[:], lhsT=qsT_sb[:, 0:128],
                             rhs=garbage[:], start=True, stop=True)

        maxcols = const.tile([128, MG * DOCS_PER_CORE], F32)

        for dp in range(DOCS_PER_CORE // 2):
            for mg in range(MG):
                lhsT = qsT_sb[:, mg * 128:(mg + 1) * 128]
                s2 = psumS.tile([128, 2 * NPAIR], F32, tag="s")
                for h in range(2):
                    dloc = 2 * dp + h
                    sl = slice(dloc * NPAIR, (dloc + 1) * NPAIR)
                    sb = s2[:, h * NPAIR:(h + 1) * NPAIR]
                    nc.tensor.matmul(sb, lhsT=lhsT,
                                     rhs=psP_sb[:, sl], start=True,
                                     stop=False, skip_group_check=True)
                    dt = psumD.tile([128, NPAIR], F32, tag="d")
                    nc.tensor.matmul(dt[:], lhsT=lhsT,
                                     rhs=psM_sb[:, sl], start=True,
                                     stop=True, skip_group_check=True)
                    a = stage.tile([128, NPAIR], F16)
                    nc.scalar.activation(a[:], dt[:],
                                         mybir.ActivationFunctionType.Abs)
                    nc.tensor.matmul(sb, lhsT=ident_sb[:],
                                     rhs=a[:], start=False, stop=True,
                                     skip_group_check=True)
                col = mg * DOCS_PER_CORE + 2 * dp
                nc.vector.reduce_max(
                    maxcols[:, col:col + 2],
                    s2[:].rearrange("p (h n) -> p h n", h=2),
                    axis=mybir.AxisListType.X)

        fin = psumS.tile([QPG, MG * DOCS_PER_CORE], F32, tag="s")
        nc.tensor.matmul(fin[:], lhsT=ones_sb[:], rhs=maxcols[:],
                         start=True, stop=True)
        out_sb = const.tile([QPG, MG * DOCS_PER_CORE], F32)
        nc.vector.tensor_copy(out_sb[:], fin[:])

        out_r = out.rearrange("(mg q) d -> q mg d", q=QPG)
        src = out_sb[:].rearrange("q (mg d) -> q mg d", d=DOCS_PER_CORE)
        nc.sync.dma_start(out_r, src)

    return nc


def _build_direct_module():
    """Exact-fp32 fallback: fp32 matmuls + DVE reduce_max from PSUM."""
    nc = bass.Bass("TRN2", target_bir_lowering=False, debug=False)

    qsT = nc.dram_tensor("qsT", [D, QROWS], F32, kind="ExternalInput").ap()
    psT = nc.dram_tensor("psT", [D, DOCS_PER_CORE * TD], F32,
                         kind="ExternalInput").ap()
    ones = nc.dram_tensor("ones", [128, QPG], F32, kind="ExternalInput").ap()
    out = nc.dram_tensor("out", [NQ, DOCS_PER_CORE], F32,
                         kind="ExternalOutput").ap()

    with tile.TileContext(nc) as tc, ExitStack() as ctx:
        const = ctx.enter_context(tc.tile_pool(name="const", bufs=1))
        psum = ctx.enter_context(tc.tile_pool(name="psum", bufs=3, space="PSUM"))
        psum_fin = ctx.enter_context(
            tc.tile_pool(name="psum_fin", bufs=1, space="PSUM"))

        qsT_sb = const.tile([D, QROWS], F32)
        nc.sync.dma_start(qsT_sb[:], qsT[:])
        ones_sb = const.tile([128, QPG], F32)
        nc.sync.dma_start(ones_sb[:], ones[:])
        psT_sb = const.tile([D, DOCS_PER_CORE * TD], F32)
        for dloc in range(DOCS_PER_CORE):
            sl = slice(dloc * TD, (dloc + 1) * TD)
            nc.sync.dma_start(psT_sb[:, sl], psT[:, sl])

        maxcols = const.tile([128, MG * DOCS_PER_CORE], F32)

        for dloc in range(DOCS_PER_CORE):
            for mg in range(MG):
                pt = psum.tile([128, TD], F32)
                lhsT = qsT_sb[:, mg * 128:(mg + 1) * 128]
                for h in range(TD // 512):
                    nc.tensor.matmul(
                        pt[:, h * 512:(h + 1) * 512],
                        lhsT=lhsT,
                        rhs=psT_sb[:, dloc * TD + h * 512:
                                   dloc * TD + (h + 1) * 512],
                        start=True, stop=True,
                    )
                col = mg * DOCS_PER_CORE + dloc
                nc.vector.reduce_max(
                    maxcols[:, col:col + 1], pt[:],
                    axis=mybir.AxisListType.X)

        fin = psum_fin.tile([QPG, MG * DOCS_PER_CORE], F32)
        nc.tensor.matmul(fin[:], lhsT=ones_sb[:], rhs=maxcols[:],
                         start=True, stop=True)
        out_sb = const.tile([QPG, MG * DOCS_PER_CORE], F32)
        nc.vector.tensor_copy(out_sb[:], fin[:])

        out_r = out.rearrange("(mg q) d -> q mg d", q=QPG)
        src = out_sb[:].rearrange("q (mg d) -> q mg d", d=DOCS_PER_CORE)
        nc.sync.dma_start(out_r, src)

    return nc


_NC_CACHE = {}

_BUILDERS = {
    "fast": _build_fast_module,
    "pair": _build_pair_module,
    "direct": _build_direct_module,
}


def _get_nc(mode=MODE, for_sim=False):
    # The wait-split pass breaks CoreSim's scheduler bookkeeping, so sim
    # uses an unsplit build; hardware needs the split to pass walrus.
    key = (mode, for_sim)
    if key not in _NC_CACHE:
        if mode == "fast":
            nc = _build_fast_module(split_first=not for_sim)
        else:
            nc = _BUILDERS[mode]()
        if not for_sim:
            _split_multi_waits(nc)
        _NC_CACHE[key] = nc
    return _NC_CACHE[key]


def _ones_blockdiag():
    ones = np.zeros((128, QPG), dtype=np.float32)
    for q in range(QPG):
        ones[q * TQ:(q + 1) * TQ, q] = 1.0
    return ones


def _to_fp8(x):
    import ml_dtypes
    return np.clip(x, -240.0, 240.0).astype(ml_dtypes.float8_e4m3fn)


def _make_in_maps(qs, ps, mode=MODE):
    qs = np.ascontiguousarray(np.asarray(qs), dtype=np.float32)
    ps = np.ascontiguousarray(np.asarray(ps), dtype=np.float32)
    assert qs.shape == (NQ, TQ, D) and ps.shape == (ND, TD, D)
    ones = _ones_blockdiag()

    in_maps = []
    if mode in ("fast", "pair"):
        cvt = (_to_fp8 if (mode == "fast" and FDAT == FP8)
               else (lambda x: x.astype(np.float16)))
        qsT = cvt(np.ascontiguousarray(
            qs.reshape(QROWS, D).T))                            # [128, 2048]
        pe = ps[:, 0::2, :]
        po = ps[:, 1::2, :]
        pplus = (pe + po) * 0.5                                 # [64,512,128]
        pminus = (pe - po) * 0.5
        ident = np.eye(128, dtype=np.float16)
        for k in range(N_CORES):
            sh = slice(k * DOCS_PER_CORE, (k + 1) * DOCS_PER_CORE)
            pP = cvt(np.ascontiguousarray(
                pplus[sh].reshape(DOCS_PER_CORE * NPAIR, D).T))  # [128, 4096]
            pM = cvt(np.ascontiguousarray(
                pminus[sh].reshape(DOCS_PER_CORE * NPAIR, D).T))
            in_maps.append({"qsT": qsT, "psP": pP, "psM": pM,
                            "ident": ident, "ones": ones})
    else:
        qsT = np.ascontiguousarray(qs.reshape(QROWS, D).T)      # [128, 2048]
        for k in range(N_CORES):
            shard = ps[k * DOCS_PER_CORE:(k + 1) * DOCS_PER_CORE]
            psTk = np.ascontiguousarray(
                shard.reshape(DOCS_PER_CORE * TD, D).T)
            in_maps.append({"qsT": qsT, "psT": psTk, "ones": ones})
    return in_maps


def _gather(results):
    return np.concatenate(
        [results[k]["out"] for k in range(N_CORES)], axis=1)


def kernel(qs, ps):
    nc = _get_nc()
    in_maps = _make_in_maps(qs, ps)
    res = bass_utils.run_bass_kernel_spmd(
        nc, in_maps, core_ids=list(range(N_CORES)))
    return _gather(res.results)


def kernel_timed(qs, ps, trace_cores=None):
    """Run with NTFF tracing; returns (scores, BassKernelResults)."""
    nc = _get_nc()
    in_maps = _make_in_maps(qs, ps)
    res = bass_utils.run_bass_kernel_spmd(
        nc, in_maps, core_ids=list(range(N_CORES)), trace=True,
        trace_cores=trace_cores)
    return _gather(res.results), res



# revision 8
# speedup vs baseline: 1.0247x; 1.0247x over previous
"""ColBERT MaxSim kernel for 8 Trainium2 NeuronCores.

scores[b, c] = sum_n max_s (qs[b, n, :] . ps[c, s, :])
  qs: (64, 32, 128) f32, ps: (64, 1024, 128) f32 -> scores: (64, 64) f32

Sharding: docs (c) are sharded 8 per core; qs is replicated. Each core
computes its (64, 8) score tile; the host concatenates along c.

Mode "fast" (default) per-core dataflow:
  - Doc tokens are combined in PAIRS on the host: P+ = (Pe+Po)/2,
    P- = (Pe-Po)/2, so max(a,b) = S + |D| with S = Q.P+, D = Q.P-.
  - The kernel is PSUM-drain-bound: every sim-derivative must cross
    PSUM->engine at 1 elem/lane/cycle, and only ScalarE (1.2 GHz) and
    VectorE (0.96 GHz) can read PSUM (DMA/GpSimd have no PSUM route).
    Per (M-group, doc) tile the drain is 512 (D, via ScalarE Abs) +
    512 (S, via VectorE reduce_max) - an even, optimal 2-engine split.
  - Docs are processed in PAIRS: one ACTIVATE Abs over the two D banks
    [128, 1024] (~1110 ns, vs 2x690 unbatched) and one reduce_max over
    [128, 2, 512] (~1224 ns). The steady-state period is then bound by
    the PE at 6 x 216 ns = ~1295 ns per 2-doc group (4 data matmuls +
    2 identity folds), with ScalarE/VectorE just under it.
  - The D matmuls are emitted under tc.high_priority(offset=8): the
    Tile scheduler otherwise hoists the S matmuls (which wait on the
    2-generations-back reduce) ahead of them, starving the Abs and
    inflating the period ~20%. Offset 8 gives exactly one group of
    D-lookahead (16 let two groups jump the first identity folds,
    stretching pipeline fill by ~1.7us).
  - fp16 everywhere: fp8 operands trigger a ~1.2x whole-core clock
    throttle (measured: every engine slows 20% when fp8 matmuls are in
    the stream), and fp8 DoubleRow additionally loses FWL. fp16 rel
    err vs the f32 reference: ~3e-5 (tolerance 2e-2).
  - Head: ~7.5 us NEFF preamble (fixed), then input DMA in per-doc
    first chunks split across both HWDGE queues; HAM warmup matmuls
    (PE needs ~5 us of sustained activity to lift the clock gate
    1.2 -> 2.4 GHz) bridge the preamble->data window.
  - Tail: token-sums run as float32r (single-pass) matmuls; docs 0-5
    flush mid-stream, so only the last 2 columns' fin+copy+DMA sit
    after the last reduce. A keep-alive DMA gated on the 2nd-to-last
    group's maxcols keeps the sync queue mid-stream, cutting the final
    transfer's completion latency 2.1 -> 1.5 us.

Mode "pair" is the previous all-fp16 per-doc-abs kernel, mode
"direct" the exact-fp32 fallback.
"""

import os
import sys
from contextlib import ExitStack

import numpy as np

sys.path.insert(0, "/opt/trn_rl_repo")
sys.path.insert(0, "/opt/trn_rl_repo/concourse")

import bass_rust
import concourse.bass as bass
import concourse.mybir as mybir
import concourse.tile as tile
from concourse import bass_utils

# Problem shape (hardcoded per contract)
N_CORES = 8
NQ, TQ, D = 64, 32, 128          # queries, query tokens, dim
ND, TD = 64, 1024                # docs, doc tokens
DOCS_PER_CORE = ND // N_CORES    # 8
QROWS = NQ * TQ                  # 2048 query-token rows
MG = QROWS // 128                # 16 M-groups of 128 rows
QPG = 128 // TQ                  # 4 queries per M-group
NPAIR = TD // 2                  # 512 token pairs per doc

F32 = mybir.dt.float32
F32R = mybir.dt.float32r
F16 = mybir.dt.float16
FP8 = mybir.dt.float8e4

MODE = os.environ.get("KERNEL_MODE", "fast")
FDAT = FP8 if os.environ.get("KERNEL_FP8", "0") == "1" else F16


def _split_multi_waits(nc):
    """This walrus build rejects >1 embedded sync wait per instruction
    ("Too many sync wait commands"). Split extras onto single-wait NoOps
    inserted just before the instruction on the same engine — semantically
    identical (per-engine program order is preserved)."""
    n_split = 0
    for fn in nc.m.functions:
        for blk in fn.blocks:
            out = []
            for ins in blk.instructions:
                si = ins.sync_info
                waits = list(si.on_wait) if si and si.on_wait else []
                if len(waits) > 1:
                    for j, w in enumerate(waits[:-1]):
                        nop = mybir.InstNoOp(
                            name=f"{ins.name}_sw{j}", ins=[], outs=[])
                        nop.engine = ins.engine
                        nop.sync_info = bass_rust.SyncInfo(
                            on_wait=[w], on_update=[])
                        out.append(nop)
                    ins.sync_info = bass_rust.SyncInfo(
                        on_wait=[waits[-1]], on_update=list(si.on_update))
                    n_split += 1
                out.append(ins)
            blk.instructions = out
    return n_split


def _build_fast_module(split_first=True):
    # split_first: emit the group-0 matmuls as two half-bank matmuls so
    # they gate on the half-chunk DMAs. Semantically identical; CoreSim's
    # per-bank PSUM zero-tracking rejects it, so sim builds disable it.
    nc = bass.Bass("TRN2", target_bir_lowering=False, debug=False)

    qsT = nc.dram_tensor("qsT", [D, QROWS], FDAT, kind="ExternalInput").ap()
    psP = nc.dram_tensor("psP", [D, DOCS_PER_CORE * NPAIR], FDAT,
                         kind="ExternalInput").ap()
    psM = nc.dram_tensor("psM", [D, DOCS_PER_CORE * NPAIR], FDAT,
                         kind="ExternalInput").ap()
    ident = nc.dram_tensor("ident", [128, 128], F16,
                           kind="ExternalInput").ap()
    ones = nc.dram_tensor("ones", [128, QPG], F32R,
                          kind="ExternalInput").ap()
    out = nc.dram_tensor("out", [NQ, DOCS_PER_CORE], F32,
                         kind="ExternalOutput").ap()

    with tile.TileContext(nc) as tc, ExitStack() as ctx:
        const = ctx.enter_context(tc.tile_pool(name="const", bufs=1))
        stage = ctx.enter_context(tc.tile_pool(name="stage", bufs=4))
        psumS = ctx.enter_context(
            tc.tile_pool(name="psumS", bufs=2, space="PSUM"))
        psumD = ctx.enter_context(
            tc.tile_pool(name="psumD", bufs=2, space="PSUM"))

        qsT_sb = const.tile([D, QROWS], FDAT)
        psP_sb = const.tile([D, DOCS_PER_CORE * NPAIR], FDAT)
        psM_sb = const.tile([D, DOCS_PER_CORE * NPAIR], FDAT)
        ident_sb = const.tile([128, 128], F16)
        ones_sb = const.tile([128, QPG], F32R)
        scratch = const.tile([128, QPG], F32R)

        # First chunks cover doc 0 (in halves, so the group-0 matmuls can
        # gate on quarter-transfers and dodge straggler DMA engine-streams
        # — measured: 1 of 16 streams can land ~1.7us after the other 15),
        # then doc 1, then the rest. Issues spread across THREE HWDGE
        # queues (sync + scalar + vector) to parallelize the ~650ns
        # descriptor writes and the queue spin-up.
        q0 = 256        # M-groups 0-1
        H = NPAIR // 2
        nc.sync.dma_start(qsT_sb[:, 0:q0], qsT[:, 0:q0])
        nc.scalar.dma_start(psM_sb[:, 0:H], psM[:, 0:H])
        nc.sync.dma_start(psP_sb[:, 0:H], psP[:, 0:H])
        nc.scalar.dma_start(psM_sb[:, H:NPAIR], psM[:, H:NPAIR])
        nc.sync.dma_start(psP_sb[:, H:NPAIR], psP[:, H:NPAIR])
        nc.scalar.dma_start(psP_sb[:, NPAIR:2 * NPAIR],
                            psP[:, NPAIR:2 * NPAIR])
        nc.sync.dma_start(psM_sb[:, NPAIR:2 * NPAIR],
                          psM[:, NPAIR:2 * NPAIR])
        # Prefetch the Abs ACT table set (~2.7us TABLE_LOAD + drain) NOW so
        # it overlaps the initial DMA instead of gating the first real abs.
        warm = stage.tile([1, 2], F16, tag="warm")
        nc.gpsimd.memset(warm[:], 0.0)
        warm2 = stage.tile([1, 2], F16, tag="warm2")
        nc.scalar.activation(warm2[:], warm[:],
                             mybir.ActivationFunctionType.Abs)
        nc.scalar.dma_start(ident_sb[:], ident[:])
        nc.sync.dma_start(qsT_sb[:, q0:1152], qsT[:, q0:1152])
        nc.scalar.dma_start(qsT_sb[:, 1152:], qsT[:, 1152:])
        # NOTE: do NOT route bulk through nc.gpsimd.dma_start (SWDGE):
        # measured, it throttles the whole-core clock 1.2x for the entire
        # run (1295 -> 1554 ns/group) and corrupted the transfer.
        nc.scalar.dma_start(psM_sb[:, 2 * NPAIR:], psM[:, 2 * NPAIR:])
        nc.sync.dma_start(psP_sb[:, 2 * NPAIR:], psP[:, 2 * NPAIR:])
        nc.sync.dma_start(ones_sb[:], ones[:])

        # HAM warmup: the PE needs ~5us of sustained activity to lift the
        # clock gate from 1.2 to 2.4 GHz; these matmuls bridge the NEFF
        # preamble -> first-DMA-chunk window so the real stream starts as
        # early and as warm as possible.
        garbage = const.tile([128, NPAIR], F16)
        nc.gpsimd.memset(garbage[:], 0.0)
        for _ in range(4):
            wt = psumD.tile([128, 2 * NPAIR], F32, tag="d")
            nc.tensor.matmul(wt[:, 0:384], lhsT=garbage[:, 0:128],
                             rhs=garbage[:, 0:384], start=True, stop=True)
        for _ in range(2):
            wt = psumD.tile([128, 2 * NPAIR], F32, tag="d")
            nc.tensor.matmul(wt[:, 0:384], lhsT=qsT_sb[:, 0:128],
                             rhs=garbage[:, 0:384], start=True, stop=True)

        # maxcols[p, mg*8 + dloc] = max over doc dloc's tokens for row p
        # of mg; float32r so the single-pass fin matmuls may consume it.
        maxcols = const.tile([128, MG * DOCS_PER_CORE], F32R)
        out_sb = const.tile([QPG, MG * DOCS_PER_CORE], F32)

        def emit_fin(d0, d1, m0=0, m1=MG):
            # Token-sum + copy + out-DMA for doc columns [d0, d1) of
            # M-groups [m0, m1) (float32r = single-pass matmul). Chunks
            # whose reduces are already done run mid-stream (fin borrows
            # a "d" PSUM slot between Abs consumers); only a [4, 2]
            # micro-chunk for the last group sits on the critical tail,
            # and each chunk's DMA keeps the sync queue warm for the next.
            nd = d1 - d0
            nm = m1 - m0
            mc3 = maxcols[:].rearrange("p (mg d) -> p mg d",
                                       d=DOCS_PER_CORE)
            fin = psumD.tile([QPG, nm * nd], F32, tag="d")
            nc.tensor.matmul(fin[:].rearrange("q (mg d) -> q mg d", d=nd),
                             lhsT=ones_sb[:],
                             rhs=mc3[:, m0:m1, d0:d1],
                             start=True, stop=True)
            oc = out_sb[:].rearrange("q (mg d) -> q mg d",
                                     d=DOCS_PER_CORE)[:, m0:m1, d0:d1]
            nc.vector.tensor_copy(
                oc, fin[:].rearrange("q (mg d) -> q mg d", d=nd))
            out_r = out.rearrange("(mg q) d -> q mg d", q=QPG)
            nc.sync.dma_start(
                out_r[:, m0:m1, d0:d1],
                out_sb[:].rearrange("q (mg d) -> q mg d",
                                    d=DOCS_PER_CORE)[:, m0:m1, d0:d1])

        for dp in range(DOCS_PER_CORE // 2):
            for mg in range(MG):
                if dp == 3 and mg == 6:
                    emit_fin(0, 6)
                if dp == 3 and mg == 13:
                    # keep the sync DMA queue awake for the final transfer
                    nc.sync.dma_start(scratch[:], ones[:])
                lhsT = qsT_sb[:, mg * 128:(mg + 1) * 128]
                d2 = psumD.tile([128, 2 * NPAIR], F32, tag="d")
                s2 = psumS.tile([128, 2 * NPAIR], F32, tag="s")
                # D matmuls first, at high priority: the batched Abs can
                # start as soon as both land, and never sits behind S
                # matmuls stalled on the previous reduce.
                first = split_first and dp == 0 and mg == 0
                with tc.high_priority(offset=8):
                    for h in range(2):
                        dloc = 2 * dp + h
                        sl = slice(dloc * NPAIR, (dloc + 1) * NPAIR)
                        if first and h == 0:
                            # Gate on the two half-chunk DMAs separately so
                            # a straggler stream only delays half the work.
                            nc.tensor.matmul(d2[:, 0:H], lhsT=lhsT,
                                             rhs=psM_sb[:, 0:H],
                                             start=True, stop=True,
                                             skip_group_check=True)
                            nc.tensor.matmul(d2[:, H:NPAIR], lhsT=lhsT,
                                             rhs=psM_sb[:, H:NPAIR],
                                             start=True, stop=True,
                                             skip_group_check=True)
                            continue
                        nc.tensor.matmul(d2[:, h * NPAIR:(h + 1) * NPAIR],
                                         lhsT=lhsT, rhs=psM_sb[:, sl],
                                         start=True, stop=True,
                                         skip_group_check=True)
                for h in range(2):
                    dloc = 2 * dp + h
                    sl = slice(dloc * NPAIR, (dloc + 1) * NPAIR)
                    if first and h == 0:
                        nc.tensor.matmul(s2[:, 0:H], lhsT=lhsT,
                                         rhs=psP_sb[:, 0:H],
                                         start=True, stop=False,
                                         skip_group_check=True)
                        nc.tensor.matmul(s2[:, H:NPAIR], lhsT=lhsT,
                                         rhs=psP_sb[:, H:NPAIR],
                                         start=True, stop=False,
                                         skip_group_check=True)
                        continue
                    nc.tensor.matmul(s2[:, h * NPAIR:(h + 1) * NPAIR],
                                     lhsT=lhsT, rhs=psP_sb[:, sl],
                                     start=True, stop=False,
                                     skip_group_check=True)
                # One batched Abs over both docs' D banks: fewer ACTIVATE
                # fixed costs (352 cyc each) and half the sem traffic.
                a = stage.tile([128, 2 * NPAIR], F16)
                nc.scalar.activation(a[:], d2[:],
                                     mybir.ActivationFunctionType.Abs)
                for h in range(2):
                    nc.tensor.matmul(s2[:, h * NPAIR:(h + 1) * NPAIR],
                                     lhsT=ident_sb[:],
                                     rhs=a[:, h * NPAIR:(h + 1) * NPAIR],
                                     start=False, stop=True,
                                     skip_group_check=True)
                col = mg * DOCS_PER_CORE + 2 * dp
                nc.vector.reduce_max(
                    maxcols[:, col:col + 2],
                    s2[:].rearrange("p (h n) -> p h n", h=2),
                    axis=mybir.AxisListType.X)

        # Late keep-alive: reads the 2nd-to-last group's maxcols columns,
        # so it issues ~1.3us before the final out-DMA and the sync queue
        # is mid-stream (not cold) when the last transfer arrives.
        nc.sync.dma_start(scratch[:, 0:2], maxcols[:, 118:120])
        emit_fin(6, 8)

    return nc


def _build_pair_module():
    nc = bass.Bass("TRN2", target_bir_lowering=False, debug=False)

    qsT = nc.dram_tensor("qsT", [D, QROWS], F16, kind="ExternalInput").ap()
    psP = nc.dram_tensor("psP", [D, DOCS_PER_CORE * NPAIR], F16,
                         kind="ExternalInput").ap()
    psM = nc.dram_tensor("psM", [D, DOCS_PER_CORE * NPAIR], F16,
                         kind="ExternalInput").ap()
    ident = nc.dram_tensor("ident", [128, 128], F16,
                           kind="ExternalInput").ap()
    ones = nc.dram_tensor("ones", [128, QPG], F32, kind="ExternalInput").ap()
    out = nc.dram_tensor("out", [NQ, DOCS_PER_CORE], F32,
                         kind="ExternalOutput").ap()

    with tile.TileContext(nc) as tc, ExitStack() as ctx:
        const = ctx.enter_context(tc.tile_pool(name="const", bufs=1))
        stage = ctx.enter_context(tc.tile_pool(name="stage", bufs=10))
        psumS = ctx.enter_context(
            tc.tile_pool(name="psumS", bufs=2, space="PSUM"))
        psumD = ctx.enter_context(
            tc.tile_pool(name="psumD", bufs=4, space="PSUM"))

        qsT_sb = const.tile([D, QROWS], F16)
        psP_sb = const.tile([D, DOCS_PER_CORE * NPAIR], F16)
        psM_sb = const.tile([D, DOCS_PER_CORE * NPAIR], F16)
        ident_sb = const.tile([128, 128], F16)
        ones_sb = const.tile([128, QPG], F32)
        c0 = 2 * NPAIR
        q0 = 256
        nc.sync.dma_start(qsT_sb[:, 0:q0], qsT[:, 0:q0])
        nc.scalar.dma_start(psM_sb[:, 0:c0], psM[:, 0:c0])
        nc.sync.dma_start(psP_sb[:, 0:c0], psP[:, 0:c0])
        warm = stage.tile([1, 2], F16, tag="warm")
        nc.gpsimd.memset(warm[:], 0.0)
        warm2 = stage.tile([1, 2], F16, tag="warm2")
        nc.scalar.activation(warm2[:], warm[:],
                             mybir.ActivationFunctionType.Abs)
        nc.scalar.dma_start(ident_sb[:], ident[:])
        nc.sync.dma_start(qsT_sb[:, q0:], qsT[:, q0:])
        nc.scalar.dma_start(psM_sb[:, c0:], psM[:, c0:])
        nc.sync.dma_start(psP_sb[:, c0:], psP[:, c0:])
        nc.sync.dma_start(ones_sb[:], ones[:])

        garbage = const.tile([128, NPAIR], F16)
        nc.gpsimd.memset(garbage[:], 0.0)
        for _ in range(12):
            wt = psumD.tile([128, NPAIR], F32, tag="d")
            nc.tensor.matmul(wt[:], lhsT=garbage[:, 0:128], rhs=garbage[:],
                             start=True, stop=True)
        for _ in range(6):
            wt = psumD.tile([128, NPAIR], F32, tag="d")
            nc.tensor.matmul(wt[:], lhsT=qsT_sb[:, 0:128],
                             rhs=garbage[:], start=True, stop=True)

        maxcols = const.tile([128, MG * DOCS_PER_CORE], F32)

        for dp in range(DOCS_PER_CORE // 2):
            for mg in range(MG):
                lhsT = qsT_sb[:, mg * 128:(mg + 1) * 128]
                s2 = psumS.tile([128, 2 * NPAIR], F32, tag="s")
                for h in range(2):
                    dloc = 2 * dp + h
                    sl = slice(dloc * NPAIR, (dloc + 1) * NPAIR)
                    sb = s2[:, h * NPAIR:(h + 1) * NPAIR]
                    nc.tensor.matmul(sb, lhsT=lhsT,
                                     rhs=psP_sb[:, sl], start=True,
                                     stop=False, skip_group_check=True)
                    dt = psumD.tile([128, NPAIR], F32, tag="d")
                    nc.tensor.matmul(dt[:], lhsT=lhsT,
                                     rhs=psM_sb[:, sl], start=True,
                                     stop=True, skip_group_check=True)
                    a = stage.tile([128, NPAIR], F16)
                    nc.scalar.activation(a[:], dt[:],
                                         mybir.ActivationFunctionType.Abs)
                    nc.tensor.matmul(sb, lhsT=ident_sb[:],
                                     rhs=a[:], start=False, stop=True,
                                     skip_group_check=True)
                col = mg * DOCS_PER_CORE + 2 * dp
                nc.vector.reduce_max(
                    maxcols[:, col:col + 2],
                    s2[:].rearrange("p (h n) -> p h n", h=2),
                    axis=mybir.AxisListType.X)

        fin = psumS.tile([QPG, MG * DOCS_PER_CORE], F32, tag="s")
        nc.tensor.matmul(fin[:], lhsT=ones_sb[:], rhs=maxcols[:],
                         start=True, stop=True)
        out_sb = const.tile([QPG, MG * DOCS_PER_CORE], F32)
        nc.vector.tensor_copy(out_sb[:], fin[:])

        out_r = out.rearrange("(mg q) d -> q mg d", q=QPG)
        src = out_sb[:].rearrange("q (mg d) -> q mg d", d=DOCS_PER_CORE)
        nc.sync.dma_start(out_r, src)

    return nc


def _build_direct_module():
    """Exact-fp32 fallback: fp32 matmuls + DVE reduce_max from PSUM."""
    nc = bass.Bass("TRN2", target_bir_lowering=False, debug=False)

    qsT = nc.dram_tensor("qsT", [D, QROWS], F32, kind="ExternalInput").ap()
    psT = nc.dram_tensor("psT", [D, DOCS_PER_CORE * TD], F32,
                         kind="ExternalInput").ap()
    ones = nc.dram_tensor("ones", [128, QPG], F32, kind="ExternalInput").ap()
    out = nc.dram_tensor("out", [NQ, DOCS_PER_CORE], F32,
                         kind="ExternalOutput").ap()

    with tile.TileContext(nc) as tc, ExitStack() as ctx:
        const = ctx.enter_context(tc.tile_pool(name="const", bufs=1))
        psum = ctx.enter_context(tc.tile_pool(name="psum", bufs=3, space="PSUM"))
        psum_fin = ctx.enter_context(
            tc.tile_pool(name="psum_fin", bufs=1, space="PSUM"))

        qsT_sb = const.tile([D, QROWS], F32)
        nc.sync.dma_start(qsT_sb[:], qsT[:])
        ones_sb = const.tile([128, QPG], F32)
        nc.sync.dma_start(ones_sb[:], ones[:])
        psT_sb = const.tile([D, DOCS_PER_CORE * TD], F32)
        for dloc in range(DOCS_PER_CORE):
            sl = slice(dloc * TD, (dloc + 1) * TD)
            nc.sync.dma_start(psT_sb[:, sl], psT[:, sl])

        maxcols = const.tile([128, MG * DOCS_PER_CORE], F32)

        for dloc in range(DOCS_PER_CORE):
            for mg in range(MG):
                pt = psum.tile([128, TD], F32)
                lhsT = qsT_sb[:, mg * 128:(mg + 1) * 128]
                for h in range(TD // 512):
                    nc.tensor.matmul(
                        pt[:, h * 512:(h + 1) * 512],
                        lhsT=lhsT,
                        rhs=psT_sb[:, dloc * TD + h * 512:
                                   dloc * TD + (h + 1) * 512],
                        start=True, stop=True,
                    )
                col = mg * DOCS_PER_CORE + dloc
                nc.vector.reduce_max(
                    maxcols[:, col:col + 1], pt[:],
                    axis=mybir.AxisListType.X)

        fin = psum_fin.tile([QPG, MG * DOCS_PER_CORE], F32)
        nc.tensor.matmul(fin[:], lhsT=ones_sb[:], rhs=maxcols[:],
                         start=True, stop=True)
        out_sb = const.tile([QPG, MG * DOCS_PER_CORE], F32)
        nc.vector.tensor_copy(out_sb[:], fin[:])

        out_r = out.rearrange("(mg q) d -> q mg d", q=QPG)
        src = out_sb[:].rearrange("q (mg d) -> q mg d", d=DOCS_PER_CORE)
        nc.sync.dma_start(out_r, src)

    return nc


_NC_CACHE = {}

_BUILDERS = {
    "fast": _build_fast_module,
    "pair": _build_pair_module,
    "direct": _build_direct_module,
}


def _get_nc(mode=MODE, for_sim=False):
    # The wait-split pass breaks CoreSim's scheduler bookkeeping, so sim
    # uses an unsplit build; hardware needs the split to pass walrus.
    key = (mode, for_sim)
    if key not in _NC_CACHE:
        if mode == "fast":
            nc = _build_fast_module(split_first=not for_sim)
        else:
            nc = _BUILDERS[mode]()
        if not for_sim:
            _split_multi_waits(nc)
        _NC_CACHE[key] = nc
    return _NC_CACHE[key]


def _ones_blockdiag():
    ones = np.zeros((128, QPG), dtype=np.float32)
    for q in range(QPG):
        ones[q * TQ:(q + 1) * TQ, q] = 1.0
    return ones


def _to_fp8(x):
    import ml_dtypes
    return np.clip(x, -240.0, 240.0).astype(ml_dtypes.float8_e4m3fn)


def _make_in_maps(qs, ps, mode=MODE):
    qs = np.ascontiguousarray(np.asarray(qs), dtype=np.float32)
    ps = np.ascontiguousarray(np.asarray(ps), dtype=np.float32)
    assert qs.shape == (NQ, TQ, D) and ps.shape == (ND, TD, D)
    ones = _ones_blockdiag()

    in_maps = []
    if mode in ("fast", "pair"):
        cvt = (_to_fp8 if (mode == "fast" and FDAT == FP8)
               else (lambda x: x.astype(np.float16)))
        qsT = cvt(np.ascontiguousarray(
            qs.reshape(QROWS, D).T))                            # [128, 2048]
        pe = ps[:, 0::2, :]
        po = ps[:, 1::2, :]
        pplus = (pe + po) * 0.5                                 # [64,512,128]
        pminus = (pe - po) * 0.5
        ident = np.eye(128, dtype=np.float16)
        for k in range(N_CORES):
            sh = slice(k * DOCS_PER_CORE, (k + 1) * DOCS_PER_CORE)
            pP = cvt(np.ascontiguousarray(
                pplus[sh].reshape(DOCS_PER_CORE * NPAIR, D).T))  # [128, 4096]
            pM = cvt(np.ascontiguousarray(
                pminus[sh].reshape(DOCS_PER_CORE * NPAIR, D).T))
            in_maps.append({"qsT": qsT, "psP": pP, "psM": pM,
                            "ident": ident, "ones": ones})
    else:
        qsT = np.ascontiguousarray(qs.reshape(QROWS, D).T)      # [128, 2048]
        for k in range(N_CORES):
            shard = ps[k * DOCS_PER_CORE:(k + 1) * DOCS_PER_CORE]
            psTk = np.ascontiguousarray(
                shard.reshape(DOCS_PER_CORE * TD, D).T)
            in_maps.append({"qsT": qsT, "psT": psTk, "ones": ones})
    return in_maps


def _gather(results):
    return np.concatenate(
        [results[k]["out"] for k in range(N_CORES)], axis=1)


def kernel(qs, ps):
    nc = _get_nc()
    in_maps = _make_in_maps(qs, ps)
    res = bass_utils.run_bass_kernel_spmd(
        nc, in_maps, core_ids=list(range(N_CORES)))
    return _gather(res.results)


def kernel_timed(qs, ps, trace_cores=None):
    """Run with NTFF tracing; returns (scores, BassKernelResults)."""
    nc = _get_nc()
    in_maps = _make_in_maps(qs, ps)
    res = bass_utils.run_bass_kernel_spmd(
        nc, in_maps, core_ids=list(range(N_CORES)), trace=True,
        trace_cores=trace_cores)
    return _gather(res.results), res



# revision 9
# speedup vs baseline: 1.2130x; 1.1837x over previous
"""ColBERT MaxSim kernel for 8 Trainium2 NeuronCores.

scores[b, c] = sum_n max_s (qs[b, n, :] . ps[c, s, :])
  qs: (64, 32, 128) f32, ps: (64, 1024, 128) f32 -> scores: (64, 64) f32

Sharding: docs (c) are sharded 8 per core; qs is replicated. Each core
computes its (64, 8) score tile; the host concatenates along c.

Mode "fast" (default) per-core dataflow:
  - Doc tokens are combined in PAIRS on the host: P+ = (Pe+Po)/2,
    P- = (Pe-Po)/2, so max(a,b) = S + |D| with S = Q.P+, D = Q.P-.
  - The kernel is PSUM-drain-bound: every sim-derivative must cross
    PSUM->engine at 1 elem/lane/cycle, and only ScalarE (1.2 GHz) and
    VectorE (0.96 GHz) can read PSUM (DMA/GpSimd have no PSUM route).
    Per (M-group, doc) tile the drain is 512 (D, via ScalarE Abs) +
    512 (S, via VectorE reduce_max) - an even, optimal 2-engine split.
  - Docs are processed in PAIRS: one ACTIVATE Abs over the two D banks
    [128, 1024] (~1110 ns, vs 2x690 unbatched) and one reduce_max over
    [128, 2, 512] (~1224 ns). The steady-state period is then bound by
    the PE at 6 x 216 ns = ~1295 ns per 2-doc group (4 data matmuls +
    2 identity folds), with ScalarE/VectorE just under it.
  - The D matmuls are emitted under tc.high_priority(offset=8): the
    Tile scheduler otherwise hoists the S matmuls (which wait on the
    2-generations-back reduce) ahead of them, starving the Abs and
    inflating the period ~20%. Offset 8 gives exactly one group of
    D-lookahead (16 let two groups jump the first identity folds,
    stretching pipeline fill by ~1.7us).
  - fp16 everywhere: fp8 operands trigger a ~1.2x whole-core clock
    throttle (measured: every engine slows 20% when fp8 matmuls are in
    the stream), and fp8 DoubleRow additionally loses FWL. fp16 rel
    err vs the f32 reference: ~3e-5 (tolerance 2e-2).
  - Head: ~7.5 us NEFF preamble (fixed), then input DMA in per-doc
    first chunks split across both HWDGE queues; HAM warmup matmuls
    (PE needs ~5 us of sustained activity to lift the clock gate
    1.2 -> 2.4 GHz) bridge the preamble->data window.
  - Tail: token-sums run as float32r (single-pass) matmuls; docs 0-5
    flush mid-stream, so only the last 2 columns' fin+copy+DMA sit
    after the last reduce. A keep-alive DMA gated on the 2nd-to-last
    group's maxcols keeps the sync queue mid-stream, cutting the final
    transfer's completion latency 2.1 -> 1.5 us.

Mode "pair" is the previous all-fp16 per-doc-abs kernel, mode
"direct" the exact-fp32 fallback.
"""

import os
import sys
from contextlib import ExitStack

import numpy as np

sys.path.insert(0, "/opt/trn_rl_repo")
sys.path.insert(0, "/opt/trn_rl_repo/concourse")

import bass_rust
import concourse.bass as bass
import concourse.mybir as mybir
import concourse.tile as tile
from concourse import bass_utils

# Problem shape (hardcoded per contract)
N_CORES = 8
NQ, TQ, D = 64, 32, 128          # queries, query tokens, dim
ND, TD = 64, 1024                # docs, doc tokens
DOCS_PER_CORE = ND // N_CORES    # 8
QROWS = NQ * TQ                  # 2048 query-token rows
MG = QROWS // 128                # 16 M-groups of 128 rows
QPG = 128 // TQ                  # 4 queries per M-group
NPAIR = TD // 2                  # 512 token pairs per doc

F32 = mybir.dt.float32
F32R = mybir.dt.float32r
F16 = mybir.dt.float16
FP8 = mybir.dt.float8e4

MODE = os.environ.get("KERNEL_MODE", "fast")
FDAT = FP8 if os.environ.get("KERNEL_FP8", "0") == "1" else F16


def _split_multi_waits(nc):
    """This walrus build rejects >1 embedded sync wait per instruction
    ("Too many sync wait commands"). Split extras onto single-wait NoOps
    inserted just before the instruction on the same engine — semantically
    identical (per-engine program order is preserved)."""
    n_split = 0
    for fn in nc.m.functions:
        for blk in fn.blocks:
            out = []
            for ins in blk.instructions:
                si = ins.sync_info
                waits = list(si.on_wait) if si and si.on_wait else []
                if len(waits) > 1:
                    for j, w in enumerate(waits[:-1]):
                        nop = mybir.InstNoOp(
                            name=f"{ins.name}_sw{j}", ins=[], outs=[])
                        nop.engine = ins.engine
                        nop.sync_info = bass_rust.SyncInfo(
                            on_wait=[w], on_update=[])
                        out.append(nop)
                    ins.sync_info = bass_rust.SyncInfo(
                        on_wait=[waits[-1]], on_update=list(si.on_update))
                    n_split += 1
                out.append(ins)
            blk.instructions = out
    return n_split


def _build_fast_module(split_first=True):
    nc = bass.Bass("TRN2", target_bir_lowering=False, debug=False)

    qsT = nc.dram_tensor("qsT", [D, QROWS], FDAT, kind="ExternalInput").ap()
    psP = nc.dram_tensor("psP", [D, DOCS_PER_CORE * NPAIR], FDAT,
                         kind="ExternalInput").ap()
    psM = nc.dram_tensor("psM", [D, DOCS_PER_CORE * NPAIR], FDAT,
                         kind="ExternalInput").ap()
    ident = nc.dram_tensor("ident", [128, 128], F16,
                           kind="ExternalInput").ap()
    ones = nc.dram_tensor("ones", [128, QPG], F32R,
                          kind="ExternalInput").ap()
    out = nc.dram_tensor("out", [NQ, DOCS_PER_CORE], F32,
                         kind="ExternalOutput").ap()

    with tile.TileContext(nc) as tc, ExitStack() as ctx:
        const = ctx.enter_context(tc.tile_pool(name="const", bufs=1))
        stage = ctx.enter_context(tc.tile_pool(name="stage", bufs=4))
        psumS = ctx.enter_context(
            tc.tile_pool(name="psumS", bufs=2, space="PSUM"))
        psumD = ctx.enter_context(
            tc.tile_pool(name="psumD", bufs=2, space="PSUM"))

        qsT_sb = const.tile([D, QROWS], FDAT)
        psP_sb = const.tile([D, DOCS_PER_CORE * NPAIR], FDAT)
        psM_sb = const.tile([D, DOCS_PER_CORE * NPAIR], FDAT)
        ident_sb = const.tile([128, 128], F16)
        ones_sb = const.tile([128, QPG], F32R)
        scratch = const.tile([128, QPG], F32R)

        # First chunks cover doc 0, then doc 1, then the rest, so the first
        # group's D/S matmuls start as soon as possible; issues split
        # across both HWDGE engines (sync + scalar).
        q0 = 256        # M-groups 0-1
        H = NPAIR // 2
        nc.sync.dma_start(qsT_sb[:, 0:q0], qsT[:, 0:q0])
        nc.scalar.dma_start(psM_sb[:, 0:H], psM[:, 0:H])
        nc.scalar.dma_start(psM_sb[:, H:NPAIR], psM[:, H:NPAIR])
        nc.sync.dma_start(psP_sb[:, 0:H], psP[:, 0:H])
        nc.sync.dma_start(psP_sb[:, H:NPAIR], psP[:, H:NPAIR])
        nc.scalar.dma_start(psM_sb[:, NPAIR:2 * NPAIR],
                            psM[:, NPAIR:2 * NPAIR])
        nc.sync.dma_start(psP_sb[:, NPAIR:2 * NPAIR],
                          psP[:, NPAIR:2 * NPAIR])
        # Prefetch the Abs ACT table set (~2.7us TABLE_LOAD + drain) NOW so
        # it overlaps the initial DMA instead of gating the first real abs.
        warm = stage.tile([1, 2], F16, tag="warm")
        nc.gpsimd.memset(warm[:], 0.0)
        warm2 = stage.tile([1, 2], F16, tag="warm2")
        nc.scalar.activation(warm2[:], warm[:],
                             mybir.ActivationFunctionType.Abs)
        nc.scalar.dma_start(ident_sb[:], ident[:])
        nc.sync.dma_start(qsT_sb[:, q0:], qsT[:, q0:])
        nc.scalar.dma_start(psM_sb[:, 2 * NPAIR:], psM[:, 2 * NPAIR:])
        nc.sync.dma_start(psP_sb[:, 2 * NPAIR:], psP[:, 2 * NPAIR:])
        nc.sync.dma_start(ones_sb[:], ones[:])

        # HAM warmup: the PE needs ~5us of sustained activity to lift the
        # clock gate from 1.2 to 2.4 GHz; these matmuls bridge the NEFF
        # preamble -> first-DMA-chunk window so the real stream starts as
        # early and as warm as possible.
        garbage = const.tile([128, NPAIR], F16)
        nc.gpsimd.memset(garbage[:], 0.0)
        for _ in range(5):
            wt = psumD.tile([128, 2 * NPAIR], F32, tag="d")
            nc.tensor.matmul(wt[:, 0:NPAIR], lhsT=garbage[:, 0:128],
                             rhs=garbage[:], start=True, stop=True)
        for _ in range(2):
            wt = psumD.tile([128, 2 * NPAIR], F32, tag="d")
            nc.tensor.matmul(wt[:, 0:NPAIR], lhsT=qsT_sb[:, 0:128],
                             rhs=garbage[:], start=True, stop=True)
        # Short filler warmups: keep the PE busy with ~280ns quanta right up
        # to first-chunk arrival. A multi-us PE hole during the ramp window
        # was observed to keep the whole core at 2.0 GHz for the entire run.
        for _ in range(3):
            wt = psumD.tile([128, 2 * NPAIR], F32, tag="d")
            nc.tensor.matmul(wt[:, 0:256], lhsT=qsT_sb[:, 0:128],
                             rhs=garbage[:, 0:256], start=True, stop=True)

        # maxcols[p, mg*8 + dloc] = max over doc dloc's tokens for row p
        # of mg; float32r so the single-pass fin matmuls may consume it.
        maxcols = const.tile([128, MG * DOCS_PER_CORE], F32R)
        out_sb = const.tile([QPG, MG * DOCS_PER_CORE], F32)

        def emit_fin(d0, d1, m0=0, m1=MG):
            # Token-sum + copy + out-DMA for doc columns [d0, d1) of
            # M-groups [m0, m1) (float32r = single-pass matmul). Chunks
            # whose reduces are already done run mid-stream (fin borrows
            # a "d" PSUM slot between Abs consumers); only a [4, 2]
            # micro-chunk for the last group sits on the critical tail,
            # and each chunk's DMA keeps the sync queue warm for the next.
            nd = d1 - d0
            nm = m1 - m0
            mc3 = maxcols[:].rearrange("p (mg d) -> p mg d",
                                       d=DOCS_PER_CORE)
            fin = psumD.tile([QPG, nm * nd], F32, tag="d")
            nc.tensor.matmul(fin[:].rearrange("q (mg d) -> q mg d", d=nd),
                             lhsT=ones_sb[:],
                             rhs=mc3[:, m0:m1, d0:d1],
                             start=True, stop=True)
            oc = out_sb[:].rearrange("q (mg d) -> q mg d",
                                     d=DOCS_PER_CORE)[:, m0:m1, d0:d1]
            nc.vector.tensor_copy(
                oc, fin[:].rearrange("q (mg d) -> q mg d", d=nd))
            out_r = out.rearrange("(mg q) d -> q mg d", q=QPG)
            nc.sync.dma_start(
                out_r[:, m0:m1, d0:d1],
                out_sb[:].rearrange("q (mg d) -> q mg d",
                                    d=DOCS_PER_CORE)[:, m0:m1, d0:d1])

        for dp in range(DOCS_PER_CORE // 2):
            for mg in range(MG):
                if dp == 3 and mg == 6:
                    emit_fin(0, 6)
                if dp == 3 and mg == 13:
                    # keep the sync DMA queue awake for the final transfer
                    nc.sync.dma_start(scratch[:], ones[:])
                lhsT = qsT_sb[:, mg * 128:(mg + 1) * 128]
                d2 = psumD.tile([128, 2 * NPAIR], F32, tag="d")
                s2 = psumS.tile([128, 2 * NPAIR], F32, tag="s")
                # D matmuls first, at high priority: the batched Abs can
                # start as soon as both land, and never sits behind S
                # matmuls stalled on the previous reduce.
                first = split_first and dp == 0 and mg == 0
                with tc.high_priority(offset=8):
                    for h in range(2):
                        dloc = 2 * dp + h
                        sl = slice(dloc * NPAIR, (dloc + 1) * NPAIR)
                        if first and h == 0:
                            # Halves gate on the half-chunk DMAs. start=True
                            # zeroes the WHOLE bank, so half1 must run with
                            # start=False (accumulate onto the zeroed bank).
                            nc.tensor.matmul(d2[:, 0:H], lhsT=lhsT,
                                             rhs=psM_sb[:, 0:H],
                                             start=True, stop=False,
                                             skip_group_check=True)
                            nc.tensor.matmul(d2[:, H:NPAIR], lhsT=lhsT,
                                             rhs=psM_sb[:, H:NPAIR],
                                             start=False, stop=True,
                                             skip_group_check=True)
                            continue
                        nc.tensor.matmul(d2[:, h * NPAIR:(h + 1) * NPAIR],
                                         lhsT=lhsT, rhs=psM_sb[:, sl],
                                         start=True, stop=True,
                                         skip_group_check=True)
                for h in range(2):
                    dloc = 2 * dp + h
                    sl = slice(dloc * NPAIR, (dloc + 1) * NPAIR)
                    if first and h == 0:
                        nc.tensor.matmul(s2[:, 0:H], lhsT=lhsT,
                                         rhs=psP_sb[:, 0:H],
                                         start=True, stop=False,
                                         skip_group_check=True)
                        nc.tensor.matmul(s2[:, H:NPAIR], lhsT=lhsT,
                                         rhs=psP_sb[:, H:NPAIR],
                                         start=False, stop=False,
                                         skip_group_check=True)
                        continue
                    nc.tensor.matmul(s2[:, h * NPAIR:(h + 1) * NPAIR],
                                     lhsT=lhsT, rhs=psP_sb[:, sl],
                                     start=True, stop=False,
                                     skip_group_check=True)
                # One batched Abs over both docs' D banks: fewer ACTIVATE
                # fixed costs (352 cyc each) and half the sem traffic.
                a = stage.tile([128, 2 * NPAIR], F16)
                nc.scalar.activation(a[:], d2[:],
                                     mybir.ActivationFunctionType.Abs)
                for h in range(2):
                    nc.tensor.matmul(s2[:, h * NPAIR:(h + 1) * NPAIR],
                                     lhsT=ident_sb[:],
                                     rhs=a[:, h * NPAIR:(h + 1) * NPAIR],
                                     start=False, stop=True,
                                     skip_group_check=True)
                col = mg * DOCS_PER_CORE + 2 * dp
                nc.vector.reduce_max(
                    maxcols[:, col:col + 2],
                    s2[:].rearrange("p (h n) -> p h n", h=2),
                    axis=mybir.AxisListType.X)

        # Late keep-alive: reads the 2nd-to-last group's maxcols columns,
        # so it issues ~1.3us before the final out-DMA and the sync queue
        # is mid-stream (not cold) when the last transfer arrives.
        nc.sync.dma_start(scratch[:, 0:2], maxcols[:, 118:120])
        emit_fin(6, 8)

    return nc


def _build_pair_module():
    nc = bass.Bass("TRN2", target_bir_lowering=False, debug=False)

    qsT = nc.dram_tensor("qsT", [D, QROWS], F16, kind="ExternalInput").ap()
    psP = nc.dram_tensor("psP", [D, DOCS_PER_CORE * NPAIR], F16,
                         kind="ExternalInput").ap()
    psM = nc.dram_tensor("psM", [D, DOCS_PER_CORE * NPAIR], F16,
                         kind="ExternalInput").ap()
    ident = nc.dram_tensor("ident", [128, 128], F16,
                           kind="ExternalInput").ap()
    ones = nc.dram_tensor("ones", [128, QPG], F32, kind="ExternalInput").ap()
    out = nc.dram_tensor("out", [NQ, DOCS_PER_CORE], F32,
                         kind="ExternalOutput").ap()

    with tile.TileContext(nc) as tc, ExitStack() as ctx:
        const = ctx.enter_context(tc.tile_pool(name="const", bufs=1))
        stage = ctx.enter_context(tc.tile_pool(name="stage", bufs=10))
        psumS = ctx.enter_context(
            tc.tile_pool(name="psumS", bufs=2, space="PSUM"))
        psumD = ctx.enter_context(
            tc.tile_pool(name="psumD", bufs=4, space="PSUM"))

        qsT_sb = const.tile([D, QROWS], F16)
        psP_sb = const.tile([D, DOCS_PER_CORE * NPAIR], F16)
        psM_sb = const.tile([D, DOCS_PER_CORE * NPAIR], F16)
        ident_sb = const.tile([128, 128], F16)
        ones_sb = const.tile([128, QPG], F32)
        c0 = 2 * NPAIR
        q0 = 256
        nc.sync.dma_start(qsT_sb[:, 0:q0], qsT[:, 0:q0])
        nc.scalar.dma_start(psM_sb[:, 0:c0], psM[:, 0:c0])
        nc.sync.dma_start(psP_sb[:, 0:c0], psP[:, 0:c0])
        warm = stage.tile([1, 2], F16, tag="warm")
        nc.gpsimd.memset(warm[:], 0.0)
        warm2 = stage.tile([1, 2], F16, tag="warm2")
        nc.scalar.activation(warm2[:], warm[:],
                             mybir.ActivationFunctionType.Abs)
        nc.scalar.dma_start(ident_sb[:], ident[:])
        nc.sync.dma_start(qsT_sb[:, q0:], qsT[:, q0:])
        nc.scalar.dma_start(psM_sb[:, c0:], psM[:, c0:])
        nc.sync.dma_start(psP_sb[:, c0:], psP[:, c0:])
        nc.sync.dma_start(ones_sb[:], ones[:])

        garbage = const.tile([128, NPAIR], F16)
        nc.gpsimd.memset(garbage[:], 0.0)
        for _ in range(12):
            wt = psumD.tile([128, NPAIR], F32, tag="d")
            nc.tensor.matmul(wt[:], lhsT=garbage[:, 0:128], rhs=garbage[:],
                             start=True, stop=True)
        for _ in range(6):
            wt = psumD.tile([128, NPAIR], F32, tag="d")
            nc.tensor.matmul(wt[:], lhsT=qsT_sb[:, 0:128],
                             rhs=garbage[:], start=True, stop=True)

        maxcols = const.tile([128, MG * DOCS_PER_CORE], F32)

        for dp in range(DOCS_PER_CORE // 2):
            for mg in range(MG):
                lhsT = qsT_sb[:, mg * 128:(mg + 1) * 128]
                s2 = psumS.tile([128, 2 * NPAIR], F32, tag="s")
                for h in range(2):
                    dloc = 2 * dp + h
                    sl = slice(dloc * NPAIR, (dloc + 1) * NPAIR)
                    sb = s2[:, h * NPAIR:(h + 1) * NPAIR]
                    nc.tensor.matmul(sb, lhsT=lhsT,
                                     rhs=psP_sb[:, sl], start=True,
                                     stop=False, skip_group_check=True)
                    dt = psumD.tile([128, NPAIR], F32, tag="d")
                    nc.tensor.matmul(dt[:], lhsT=lhsT,
                                     rhs=psM_sb[:, sl], start=True,
                                     stop=True, skip_group_check=True)
                    a = stage.tile([128, NPAIR], F16)
                    nc.scalar.activation(a[:], dt[:],
                                         mybir.ActivationFunctionType.Abs)
                    nc.tensor.matmul(sb, lhsT=ident_sb[:],
                                     rhs=a[:], start=False, stop=True,
                                     skip_group_check=True)
                col = mg * DOCS_PER_CORE + 2 * dp
                nc.vector.reduce_max(
                    maxcols[:, col:col + 2],
                    s2[:].rearrange("p (h n) -> p h n", h=2),
                    axis=mybir.AxisListType.X)

        fin = psumS.tile([QPG, MG * DOCS_PER_CORE], F32, tag="s")
        nc.tensor.matmul(fin[:], lhsT=ones_sb[:], rhs=maxcols[:],
                         start=True, stop=True)
        out_sb = const.tile([QPG, MG * DOCS_PER_CORE], F32)
        nc.vector.tensor_copy(out_sb[:], fin[:])

        out_r = out.rearrange("(mg q) d -> q mg d", q=QPG)
        src = out_sb[:].rearrange("q (mg d) -> q mg d", d=DOCS_PER_CORE)
        nc.sync.dma_start(out_r, src)

    return nc


def _build_direct_module():
    """Exact-fp32 fallback: fp32 matmuls + DVE reduce_max from PSUM."""
    nc = bass.Bass("TRN2", target_bir_lowering=False, debug=False)

    qsT = nc.dram_tensor("qsT", [D, QROWS], F32, kind="ExternalInput").ap()
    psT = nc.dram_tensor("psT", [D, DOCS_PER_CORE * TD], F32,
                         kind="ExternalInput").ap()
    ones = nc.dram_tensor("ones", [128, QPG], F32, kind="ExternalInput").ap()
    out = nc.dram_tensor("out", [NQ, DOCS_PER_CORE], F32,
                         kind="ExternalOutput").ap()

    with tile.TileContext(nc) as tc, ExitStack() as ctx:
        const = ctx.enter_context(tc.tile_pool(name="const", bufs=1))
        psum = ctx.enter_context(tc.tile_pool(name="psum", bufs=3, space="PSUM"))
        psum_fin = ctx.enter_context(
            tc.tile_pool(name="psum_fin", bufs=1, space="PSUM"))

        qsT_sb = const.tile([D, QROWS], F32)
        nc.sync.dma_start(qsT_sb[:], qsT[:])
        ones_sb = const.tile([128, QPG], F32)
        nc.sync.dma_start(ones_sb[:], ones[:])
        psT_sb = const.tile([D, DOCS_PER_CORE * TD], F32)
        for dloc in range(DOCS_PER_CORE):
            sl = slice(dloc * TD, (dloc + 1) * TD)
            nc.sync.dma_start(psT_sb[:, sl], psT[:, sl])

        maxcols = const.tile([128, MG * DOCS_PER_CORE], F32)

        for dloc in range(DOCS_PER_CORE):
            for mg in range(MG):
                pt = psum.tile([128, TD], F32)
                lhsT = qsT_sb[:, mg * 128:(mg + 1) * 128]
                for h in range(TD // 512):
                    nc.tensor.matmul(
                        pt[:, h * 512:(h + 1) * 512],
                        lhsT=lhsT,
                        rhs=psT_sb[:, dloc * TD + h * 512:
                                   dloc * TD + (h + 1) * 512],
                        start=True, stop=True,
                    )
                col = mg * DOCS_PER_CORE + dloc
                nc.vector.reduce_max(
                    maxcols[:, col:col + 1], pt[:],
                    axis=mybir.AxisListType.X)

        fin = psum_fin.tile([QPG, MG * DOCS_PER_CORE], F32)
        nc.tensor.matmul(fin[:], lhsT=ones_sb[:], rhs=maxcols[:],
                         start=True, stop=True)
        out_sb = const.tile([QPG, MG * DOCS_PER_CORE], F32)
        nc.vector.tensor_copy(out_sb[:], fin[:])

        out_r = out.rearrange("(mg q) d -> q mg d", q=QPG)
        src = out_sb[:].rearrange("q (mg d) -> q mg d", d=DOCS_PER_CORE)
        nc.sync.dma_start(out_r, src)

    return nc


_NC_CACHE = {}

_BUILDERS = {
    "fast": _build_fast_module,
    "pair": _build_pair_module,
    "direct": _build_direct_module,
}


def _get_nc(mode=MODE, for_sim=False):
    # The wait-split pass breaks CoreSim's scheduler bookkeeping, so sim
    # uses an unsplit build; hardware needs the split to pass walrus.
    key = (mode, for_sim)
    if key not in _NC_CACHE:
        if mode == "fast":
            # CoreSim's per-bank PSUM zero-tracking rejects the half-bank
            # group-0 matmuls; they are semantically identical, so sim
            # builds disable them.
            nc = _build_fast_module(split_first=not for_sim)
        else:
            nc = _BUILDERS[mode]()
        if not for_sim:
            _split_multi_waits(nc)
        _NC_CACHE[key] = nc
    return _NC_CACHE[key]


def _ones_blockdiag():
    ones = np.zeros((128, QPG), dtype=np.float32)
    for q in range(QPG):
        ones[q * TQ:(q + 1) * TQ, q] = 1.0
    return ones


def _to_fp8(x):
    import ml_dtypes
    return np.clip(x, -240.0, 240.0).astype(ml_dtypes.float8_e4m3fn)


def _make_in_maps(qs, ps, mode=MODE):
    qs = np.ascontiguousarray(np.asarray(qs), dtype=np.float32)
    ps = np.ascontiguousarray(np.asarray(ps), dtype=np.float32)
    assert qs.shape == (NQ, TQ, D) and ps.shape == (ND, TD, D)
    ones = _ones_blockdiag()

    in_maps = []
    if mode in ("fast", "pair"):
        cvt = (_to_fp8 if (mode == "fast" and FDAT == FP8)
               else (lambda x: x.astype(np.float16)))
        qsT = cvt(np.ascontiguousarray(
            qs.reshape(QROWS, D).T))                            # [128, 2048]
        pe = ps[:, 0::2, :]
        po = ps[:, 1::2, :]
        pplus = (pe + po) * 0.5                                 # [64,512,128]
        pminus = (pe - po) * 0.5
        ident = np.eye(128, dtype=np.float16)
        for k in range(N_CORES):
            sh = slice(k * DOCS_PER_CORE, (k + 1) * DOCS_PER_CORE)
            pP = cvt(np.ascontiguousarray(
                pplus[sh].reshape(DOCS_PER_CORE * NPAIR, D).T))  # [128, 4096]
            pM = cvt(np.ascontiguousarray(
                pminus[sh].reshape(DOCS_PER_CORE * NPAIR, D).T))
            in_maps.append({"qsT": qsT, "psP": pP, "psM": pM,
                            "ident": ident, "ones": ones})
    else:
        qsT = np.ascontiguousarray(qs.reshape(QROWS, D).T)      # [128, 2048]
        for k in range(N_CORES):
            shard = ps[k * DOCS_PER_CORE:(k + 1) * DOCS_PER_CORE]
            psTk = np.ascontiguousarray(
                shard.reshape(DOCS_PER_CORE * TD, D).T)
            in_maps.append({"qsT": qsT, "psT": psTk, "ones": ones})
    return in_maps


def _gather(results):
    return np.concatenate(
        [results[k]["out"] for k in range(N_CORES)], axis=1)


def kernel(qs, ps):
    nc = _get_nc()
    in_maps = _make_in_maps(qs, ps)
    res = bass_utils.run_bass_kernel_spmd(
        nc, in_maps, core_ids=list(range(N_CORES)))
    return _gather(res.results)


def kernel_timed(qs, ps, trace_cores=None):
    """Run with NTFF tracing; returns (scores, BassKernelResults)."""
    nc = _get_nc()
    in_maps = _make_in_maps(qs, ps)
    res = bass_utils.run_bass_kernel_spmd(
        nc, in_maps, core_ids=list(range(N_CORES)), trace=True,
        trace_cores=trace_cores)
    return _gather(res.results), res



# revision 10
# speedup vs baseline: 1.2467x; 1.0278x over previous
"""ColBERT MaxSim kernel for 8 Trainium2 NeuronCores.

scores[b, c] = sum_n max_s (qs[b, n, :] . ps[c, s, :])
  qs: (64, 32, 128) f32, ps: (64, 1024, 128) f32 -> scores: (64, 64) f32

Sharding: docs (c) are sharded 8 per core; qs is replicated. Each core
computes its (64, 8) score tile; the host concatenates along c.

Mode "fast" (default) per-core dataflow:
  - Doc tokens are combined in PAIRS on the host: P+ = (Pe+Po)/2,
    P- = (Pe-Po)/2, so max(a,b) = S + |D| with S = Q.P+, D = Q.P-.
  - The kernel is PSUM-drain-bound: every sim-derivative must cross
    PSUM->engine at 1 elem/lane/cycle, and only ScalarE (1.2 GHz) and
    VectorE (0.96 GHz) can read PSUM (DMA/GpSimd have no PSUM route).
    Per (M-group, doc) tile the drain is 512 (D, via ScalarE Abs) +
    512 (S, via VectorE reduce_max) - an even, optimal 2-engine split.
  - Docs are processed in PAIRS: one ACTIVATE Abs over the two D banks
    [128, 1024] (~1110 ns, vs 2x690 unbatched) and one reduce_max over
    [128, 2, 512] (~1224 ns). The steady-state period is then bound by
    the PE at 6 x 216 ns = ~1295 ns per 2-doc group (4 data matmuls +
    2 identity folds), with ScalarE/VectorE just under it.
  - The D matmuls are emitted under tc.high_priority(offset=8): the
    Tile scheduler otherwise hoists the S matmuls (which wait on the
    2-generations-back reduce) ahead of them, starving the Abs and
    inflating the period ~20%. Offset 8 gives exactly one group of
    D-lookahead (16 let two groups jump the first identity folds,
    stretching pipeline fill by ~1.7us).
  - fp16 everywhere: fp8 operands trigger a ~1.2x whole-core clock
    throttle (measured: every engine slows 20% when fp8 matmuls are in
    the stream), and fp8 DoubleRow additionally loses FWL. fp16 rel
    err vs the f32 reference: ~3e-5 (tolerance 2e-2).
  - Head: ~7.5 us NEFF preamble (fixed), then input DMA in per-doc
    first chunks split across both HWDGE queues; HAM warmup matmuls
    (PE needs ~5 us of sustained activity to lift the clock gate
    1.2 -> 2.4 GHz) bridge the preamble->data window.
  - Tail: token-sums run as float32r (single-pass) matmuls; docs 0-5
    flush mid-stream, so only the last 2 columns' fin+copy+DMA sit
    after the last reduce. A keep-alive DMA gated on the 2nd-to-last
    group's maxcols keeps the sync queue mid-stream, cutting the final
    transfer's completion latency 2.1 -> 1.5 us.

Mode "pair" is the previous all-fp16 per-doc-abs kernel, mode
"direct" the exact-fp32 fallback.
"""

import os
import sys
from contextlib import ExitStack

import numpy as np

sys.path.insert(0, "/opt/trn_rl_repo")
sys.path.insert(0, "/opt/trn_rl_repo/concourse")

import bass_rust
import concourse.bass as bass
import concourse.mybir as mybir
import concourse.tile as tile
from concourse import bass_utils

# Problem shape (hardcoded per contract)
N_CORES = 8
NQ, TQ, D = 64, 32, 128          # queries, query tokens, dim
ND, TD = 64, 1024                # docs, doc tokens
DOCS_PER_CORE = ND // N_CORES    # 8
QROWS = NQ * TQ                  # 2048 query-token rows
MG = QROWS // 128                # 16 M-groups of 128 rows
QPG = 128 // TQ                  # 4 queries per M-group
NPAIR = TD // 2                  # 512 token pairs per doc

F32 = mybir.dt.float32
F32R = mybir.dt.float32r
F16 = mybir.dt.float16
FP8 = mybir.dt.float8e4

MODE = os.environ.get("KERNEL_MODE", "fast")
FDAT = FP8 if os.environ.get("KERNEL_FP8", "0") == "1" else F16


def _split_multi_waits(nc):
    """This walrus build rejects >1 embedded sync wait per instruction
    ("Too many sync wait commands"). Split extras onto single-wait NoOps
    inserted just before the instruction on the same engine — semantically
    identical (per-engine program order is preserved)."""
    n_split = 0
    for fn in nc.m.functions:
        for blk in fn.blocks:
            out = []
            for ins in blk.instructions:
                si = ins.sync_info
                waits = list(si.on_wait) if si and si.on_wait else []
                if len(waits) > 1:
                    for j, w in enumerate(waits[:-1]):
                        nop = mybir.InstNoOp(
                            name=f"{ins.name}_sw{j}", ins=[], outs=[])
                        nop.engine = ins.engine
                        nop.sync_info = bass_rust.SyncInfo(
                            on_wait=[w], on_update=[])
                        out.append(nop)
                    ins.sync_info = bass_rust.SyncInfo(
                        on_wait=[waits[-1]], on_update=list(si.on_update))
                    n_split += 1
                out.append(ins)
            blk.instructions = out
    return n_split


def _build_fast_module(split_first=True):
    nc = bass.Bass("TRN2", target_bir_lowering=False, debug=False)

    qsT = nc.dram_tensor("qsT", [D, QROWS], FDAT, kind="ExternalInput").ap()
    psP = nc.dram_tensor("psP", [D, DOCS_PER_CORE * NPAIR], FDAT,
                         kind="ExternalInput").ap()
    psM = nc.dram_tensor("psM", [D, DOCS_PER_CORE * NPAIR], FDAT,
                         kind="ExternalInput").ap()
    ident = nc.dram_tensor("ident", [128, 128], F16,
                           kind="ExternalInput").ap()
    ones = nc.dram_tensor("ones", [128, QPG], F32R,
                          kind="ExternalInput").ap()
    out = nc.dram_tensor("out", [NQ, DOCS_PER_CORE], F32,
                         kind="ExternalOutput").ap()

    with tile.TileContext(nc) as tc, ExitStack() as ctx:
        const = ctx.enter_context(tc.tile_pool(name="const", bufs=1))
        stage = ctx.enter_context(tc.tile_pool(name="stage", bufs=4))
        psumS = ctx.enter_context(
            tc.tile_pool(name="psumS", bufs=2, space="PSUM"))
        psumD = ctx.enter_context(
            tc.tile_pool(name="psumD", bufs=2, space="PSUM"))

        qsT_sb = const.tile([D, QROWS], FDAT)
        psP_sb = const.tile([D, DOCS_PER_CORE * NPAIR], FDAT)
        psM_sb = const.tile([D, DOCS_PER_CORE * NPAIR], FDAT)
        ident_sb = const.tile([128, 128], F16)
        ones_sb = const.tile([128, QPG], F32R)
        scratch = const.tile([128, QPG], F32R)

        # First chunks cover doc 0, then doc 1, then the rest, so the first
        # group's D/S matmuls start as soon as possible; issues split
        # across both HWDGE engines (sync + scalar).
        q0 = 256        # M-groups 0-1
        H = NPAIR // 2
        nc.sync.dma_start(qsT_sb[:, 0:q0], qsT[:, 0:q0])
        nc.scalar.dma_start(psM_sb[:, 0:H], psM[:, 0:H])
        nc.scalar.dma_start(psM_sb[:, H:NPAIR], psM[:, H:NPAIR])
        nc.sync.dma_start(psP_sb[:, 0:H], psP[:, 0:H])
        nc.sync.dma_start(psP_sb[:, H:NPAIR], psP[:, H:NPAIR])
        nc.scalar.dma_start(psM_sb[:, NPAIR:2 * NPAIR],
                            psM[:, NPAIR:2 * NPAIR])
        nc.sync.dma_start(psP_sb[:, NPAIR:2 * NPAIR],
                          psP[:, NPAIR:2 * NPAIR])
        # Prefetch the Abs ACT table set (~2.7us TABLE_LOAD + drain) NOW so
        # it overlaps the initial DMA instead of gating the first real abs.
        warm = stage.tile([1, 2], F16, tag="warm")
        nc.gpsimd.memset(warm[:], 0.0)
        warm2 = stage.tile([1, 2], F16, tag="warm2")
        nc.scalar.activation(warm2[:], warm[:],
                             mybir.ActivationFunctionType.Abs)
        nc.scalar.dma_start(ident_sb[:], ident[:])
        nc.sync.dma_start(qsT_sb[:, q0:], qsT[:, q0:])
        nc.scalar.dma_start(psM_sb[:, 2 * NPAIR:], psM[:, 2 * NPAIR:])
        nc.sync.dma_start(psP_sb[:, 2 * NPAIR:], psP[:, 2 * NPAIR:])
        nc.sync.dma_start(ones_sb[:], ones[:])

        # HAM warmup: the PE needs ~5us of sustained activity to lift the
        # clock gate from 1.2 to 2.4 GHz; these matmuls bridge the NEFF
        # preamble -> first-DMA-chunk window so the real stream starts as
        # early and as warm as possible.
        garbage = const.tile([128, NPAIR], F16)
        nc.gpsimd.memset(garbage[:], 0.0)
        # Warmups gate on NOTHING (garbage only): in a previous variant the
        # qsT-gated warmups made the PE sit in program order behind a late
        # qsT chunk while psM data was already waiting. Short ~280ns quanta
        # near the handoff so the first data matmul slots in promptly, and
        # no multi-us PE hole opens during the clock-ramp window (a hole
        # was observed to keep the whole core at 2.0 GHz for the run).
        for _ in range(5):
            wt = psumD.tile([128, 2 * NPAIR], F32, tag="d")
            nc.tensor.matmul(wt[:, 0:NPAIR], lhsT=garbage[:, 0:128],
                             rhs=garbage[:], start=True, stop=True)
        for _ in range(4):
            wt = psumD.tile([128, 2 * NPAIR], F32, tag="d")
            nc.tensor.matmul(wt[:, 0:256], lhsT=garbage[:, 0:128],
                             rhs=garbage[:, 0:256], start=True, stop=True)

        # maxcols[p, mg*8 + dloc] = max over doc dloc's tokens for row p
        # of mg; float32r so the single-pass fin matmuls may consume it.
        maxcols = const.tile([128, MG * DOCS_PER_CORE], F32R)
        out_sb = const.tile([QPG, MG * DOCS_PER_CORE], F32)

        def emit_fin(d0, d1, m0=0, m1=MG):
            # Token-sum + copy + out-DMA for doc columns [d0, d1) of
            # M-groups [m0, m1) (float32r = single-pass matmul). Chunks
            # whose reduces are already done run mid-stream (fin borrows
            # a "d" PSUM slot between Abs consumers); only a [4, 2]
            # micro-chunk for the last group sits on the critical tail,
            # and each chunk's DMA keeps the sync queue warm for the next.
            nd = d1 - d0
            nm = m1 - m0
            mc3 = maxcols[:].rearrange("p (mg d) -> p mg d",
                                       d=DOCS_PER_CORE)
            fin = psumD.tile([QPG, nm * nd], F32, tag="d")
            nc.tensor.matmul(fin[:].rearrange("q (mg d) -> q mg d", d=nd),
                             lhsT=ones_sb[:],
                             rhs=mc3[:, m0:m1, d0:d1],
                             start=True, stop=True)
            oc = out_sb[:].rearrange("q (mg d) -> q mg d",
                                     d=DOCS_PER_CORE)[:, m0:m1, d0:d1]
            nc.vector.tensor_copy(
                oc, fin[:].rearrange("q (mg d) -> q mg d", d=nd))
            out_r = out.rearrange("(mg q) d -> q mg d", q=QPG)
            nc.sync.dma_start(
                out_r[:, m0:m1, d0:d1],
                out_sb[:].rearrange("q (mg d) -> q mg d",
                                    d=DOCS_PER_CORE)[:, m0:m1, d0:d1])

        for dp in range(DOCS_PER_CORE // 2):
            for mg in range(MG):
                if dp == 3 and mg == 6:
                    emit_fin(0, 6)
                if dp == 3 and mg == 13:
                    # keep the sync DMA queue awake for the final transfer
                    nc.sync.dma_start(scratch[:], ones[:])
                lhsT = qsT_sb[:, mg * 128:(mg + 1) * 128]
                d2 = psumD.tile([128, 2 * NPAIR], F32, tag="d")
                s2 = psumS.tile([128, 2 * NPAIR], F32, tag="s")
                # D matmuls first, at high priority: the batched Abs can
                # start as soon as both land, and never sits behind S
                # matmuls stalled on the previous reduce.
                first = split_first and dp == 0 and mg == 0
                with tc.high_priority(offset=8):
                    for h in range(2):
                        dloc = 2 * dp + h
                        sl = slice(dloc * NPAIR, (dloc + 1) * NPAIR)
                        if first and h == 0:
                            # Halves gate on the half-chunk DMAs. start=True
                            # zeroes the WHOLE bank, so half1 must run with
                            # start=False (accumulate onto the zeroed bank).
                            nc.tensor.matmul(d2[:, 0:H], lhsT=lhsT,
                                             rhs=psM_sb[:, 0:H],
                                             start=True, stop=False,
                                             skip_group_check=True)
                            nc.tensor.matmul(d2[:, H:NPAIR], lhsT=lhsT,
                                             rhs=psM_sb[:, H:NPAIR],
                                             start=False, stop=True,
                                             skip_group_check=True)
                            continue
                        nc.tensor.matmul(d2[:, h * NPAIR:(h + 1) * NPAIR],
                                         lhsT=lhsT, rhs=psM_sb[:, sl],
                                         start=True, stop=True,
                                         skip_group_check=True)
                for h in range(2):
                    dloc = 2 * dp + h
                    sl = slice(dloc * NPAIR, (dloc + 1) * NPAIR)
                    if first and h == 0:
                        nc.tensor.matmul(s2[:, 0:H], lhsT=lhsT,
                                         rhs=psP_sb[:, 0:H],
                                         start=True, stop=False,
                                         skip_group_check=True)
                        nc.tensor.matmul(s2[:, H:NPAIR], lhsT=lhsT,
                                         rhs=psP_sb[:, H:NPAIR],
                                         start=False, stop=False,
                                         skip_group_check=True)
                        continue
                    nc.tensor.matmul(s2[:, h * NPAIR:(h + 1) * NPAIR],
                                     lhsT=lhsT, rhs=psP_sb[:, sl],
                                     start=True, stop=False,
                                     skip_group_check=True)
                # One batched Abs over both docs' D banks: fewer ACTIVATE
                # fixed costs (352 cyc each) and half the sem traffic.
                a = stage.tile([128, 2 * NPAIR], F16)
                nc.scalar.activation(a[:], d2[:],
                                     mybir.ActivationFunctionType.Abs)
                for h in range(2):
                    nc.tensor.matmul(s2[:, h * NPAIR:(h + 1) * NPAIR],
                                     lhsT=ident_sb[:],
                                     rhs=a[:, h * NPAIR:(h + 1) * NPAIR],
                                     start=False, stop=True,
                                     skip_group_check=True)
                col = mg * DOCS_PER_CORE + 2 * dp
                nc.vector.reduce_max(
                    maxcols[:, col:col + 2],
                    s2[:].rearrange("p (h n) -> p h n", h=2),
                    axis=mybir.AxisListType.X)

        # Late keep-alive: reads the 2nd-to-last group's maxcols columns,
        # so it issues ~1.3us before the final out-DMA and the sync queue
        # is mid-stream (not cold) when the last transfer arrives.
        nc.sync.dma_start(scratch[:, 0:2], maxcols[:, 118:120])
        emit_fin(6, 8, 0, 14)
        emit_fin(6, 8, 14, 16)

    return nc


def _build_pair_module():
    nc = bass.Bass("TRN2", target_bir_lowering=False, debug=False)

    qsT = nc.dram_tensor("qsT", [D, QROWS], F16, kind="ExternalInput").ap()
    psP = nc.dram_tensor("psP", [D, DOCS_PER_CORE * NPAIR], F16,
                         kind="ExternalInput").ap()
    psM = nc.dram_tensor("psM", [D, DOCS_PER_CORE * NPAIR], F16,
                         kind="ExternalInput").ap()
    ident = nc.dram_tensor("ident", [128, 128], F16,
                           kind="ExternalInput").ap()
    ones = nc.dram_tensor("ones", [128, QPG], F32, kind="ExternalInput").ap()
    out = nc.dram_tensor("out", [NQ, DOCS_PER_CORE], F32,
                         kind="ExternalOutput").ap()

    with tile.TileContext(nc) as tc, ExitStack() as ctx:
        const = ctx.enter_context(tc.tile_pool(name="const", bufs=1))
        stage = ctx.enter_context(tc.tile_pool(name="stage", bufs=10))
        psumS = ctx.enter_context(
            tc.tile_pool(name="psumS", bufs=2, space="PSUM"))
        psumD = ctx.enter_context(
            tc.tile_pool(name="psumD", bufs=4, space="PSUM"))

        qsT_sb = const.tile([D, QROWS], F16)
        psP_sb = const.tile([D, DOCS_PER_CORE * NPAIR], F16)
        psM_sb = const.tile([D, DOCS_PER_CORE * NPAIR], F16)
        ident_sb = const.tile([128, 128], F16)
        ones_sb = const.tile([128, QPG], F32)
        c0 = 2 * NPAIR
        q0 = 256
        nc.sync.dma_start(qsT_sb[:, 0:q0], qsT[:, 0:q0])
        nc.scalar.dma_start(psM_sb[:, 0:c0], psM[:, 0:c0])
        nc.sync.dma_start(psP_sb[:, 0:c0], psP[:, 0:c0])
        warm = stage.tile([1, 2], F16, tag="warm")
        nc.gpsimd.memset(warm[:], 0.0)
        warm2 = stage.tile([1, 2], F16, tag="warm2")
        nc.scalar.activation(warm2[:], warm[:],
                             mybir.ActivationFunctionType.Abs)
        nc.scalar.dma_start(ident_sb[:], ident[:])
        nc.sync.dma_start(qsT_sb[:, q0:], qsT[:, q0:])
        nc.scalar.dma_start(psM_sb[:, c0:], psM[:, c0:])
        nc.sync.dma_start(psP_sb[:, c0:], psP[:, c0:])
        nc.sync.dma_start(ones_sb[:], ones[:])

        garbage = const.tile([128, NPAIR], F16)
        nc.gpsimd.memset(garbage[:], 0.0)
        for _ in range(12):
            wt = psumD.tile([128, NPAIR], F32, tag="d")
            nc.tensor.matmul(wt[:], lhsT=garbage[:, 0:128], rhs=garbage[:],
                             start=True, stop=True)
        for _ in range(6):
            wt = psumD.tile([128, NPAIR], F32, tag="d")
            nc.tensor.matmul(wt[:], lhsT=qsT_sb[:, 0:128],
                             rhs=garbage[:], start=True, stop=True)

        maxcols = const.tile([128, MG * DOCS_PER_CORE], F32)

        for dp in range(DOCS_PER_CORE // 2):
            for mg in range(MG):
                lhsT = qsT_sb[:, mg * 128:(mg + 1) * 128]
                s2 = psumS.tile([128, 2 * NPAIR], F32, tag="s")
                for h in range(2):
                    dloc = 2 * dp + h
                    sl = slice(dloc * NPAIR, (dloc + 1) * NPAIR)
                    sb = s2[:, h * NPAIR:(h + 1) * NPAIR]
                    nc.tensor.matmul(sb, lhsT=lhsT,
                                     rhs=psP_sb[:, sl], start=True,
                                     stop=False, skip_group_check=True)
                    dt = psumD.tile([128, NPAIR], F32, tag="d")
                    nc.tensor.matmul(dt[:], lhsT=lhsT,
                                     rhs=psM_sb[:, sl], start=True,
                                     stop=True, skip_group_check=True)
                    a = stage.tile([128, NPAIR], F16)
                    nc.scalar.activation(a[:], dt[:],
                                         mybir.ActivationFunctionType.Abs)
                    nc.tensor.matmul(sb, lhsT=ident_sb[:],
                                     rhs=a[:], start=False, stop=True,
                                     skip_group_check=True)
                col = mg * DOCS_PER_CORE + 2 * dp
                nc.vector.reduce_max(
                    maxcols[:, col:col + 2],
                    s2[:].rearrange("p (h n) -> p h n", h=2),
                    axis=mybir.AxisListType.X)

        fin = psumS.tile([QPG, MG * DOCS_PER_CORE], F32, tag="s")
        nc.tensor.matmul(fin[:], lhsT=ones_sb[:], rhs=maxcols[:],
                         start=True, stop=True)
        out_sb = const.tile([QPG, MG * DOCS_PER_CORE], F32)
        nc.vector.tensor_copy(out_sb[:], fin[:])

        out_r = out.rearrange("(mg q) d -> q mg d", q=QPG)
        src = out_sb[:].rearrange("q (mg d) -> q mg d", d=DOCS_PER_CORE)
        nc.sync.dma_start(out_r, src)

    return nc


def _build_direct_module():
    """Exact-fp32 fallback: fp32 matmuls + DVE reduce_max from PSUM."""
    nc = bass.Bass("TRN2", target_bir_lowering=False, debug=False)

    qsT = nc.dram_tensor("qsT", [D, QROWS], F32, kind="ExternalInput").ap()
    psT = nc.dram_tensor("psT", [D, DOCS_PER_CORE * TD], F32,
                         kind="ExternalInput").ap()
    ones = nc.dram_tensor("ones", [128, QPG], F32, kind="ExternalInput").ap()
    out = nc.dram_tensor("out", [NQ, DOCS_PER_CORE], F32,
                         kind="ExternalOutput").ap()

    with tile.TileContext(nc) as tc, ExitStack() as ctx:
        const = ctx.enter_context(tc.tile_pool(name="const", bufs=1))
        psum = ctx.enter_context(tc.tile_pool(name="psum", bufs=3, space="PSUM"))
        psum_fin = ctx.enter_context(
            tc.tile_pool(name="psum_fin", bufs=1, space="PSUM"))

        qsT_sb = const.tile([D, QROWS], F32)
        nc.sync.dma_start(qsT_sb[:], qsT[:])
        ones_sb = const.tile([128, QPG], F32)
        nc.sync.dma_start(ones_sb[:], ones[:])
        psT_sb = const.tile([D, DOCS_PER_CORE * TD], F32)
        for dloc in range(DOCS_PER_CORE):
            sl = slice(dloc * TD, (dloc + 1) * TD)
            nc.sync.dma_start(psT_sb[:, sl], psT[:, sl])

        maxcols = const.tile([128, MG * DOCS_PER_CORE], F32)

        for dloc in range(DOCS_PER_CORE):
            for mg in range(MG):
                pt = psum.tile([128, TD], F32)
                lhsT = qsT_sb[:, mg * 128:(mg + 1) * 128]
                for h in range(TD // 512):
                    nc.tensor.matmul(
                        pt[:, h * 512:(h + 1) * 512],
                        lhsT=lhsT,
                        rhs=psT_sb[:, dloc * TD + h * 512:
                                   dloc * TD + (h + 1) * 512],
                        start=True, stop=True,
                    )
                col = mg * DOCS_PER_CORE + dloc
                nc.vector.reduce_max(
                    maxcols[:, col:col + 1], pt[:],
                    axis=mybir.AxisListType.X)

        fin = psum_fin.tile([QPG, MG * DOCS_PER_CORE], F32)
        nc.tensor.matmul(fin[:], lhsT=ones_sb[:], rhs=maxcols[:],
                         start=True, stop=True)
        out_sb = const.tile([QPG, MG * DOCS_PER_CORE], F32)
        nc.vector.tensor_copy(out_sb[:], fin[:])

        out_r = out.rearrange("(mg q) d -> q mg d", q=QPG)
        src = out_sb[:].rearrange("q (mg d) -> q mg d", d=DOCS_PER_CORE)
        nc.sync.dma_start(out_r, src)

    return nc


_NC_CACHE = {}

_BUILDERS = {
    "fast": _build_fast_module,
    "pair": _build_pair_module,
    "direct": _build_direct_module,
}


def _get_nc(mode=MODE, for_sim=False):
    # The wait-split pass breaks CoreSim's scheduler bookkeeping, so sim
    # uses an unsplit build; hardware needs the split to pass walrus.
    key = (mode, for_sim)
    if key not in _NC_CACHE:
        if mode == "fast":
            # CoreSim's per-bank PSUM zero-tracking rejects the half-bank
            # group-0 matmuls; they are semantically identical, so sim
            # builds disable them.
            nc = _build_fast_module(split_first=not for_sim)
        else:
            nc = _BUILDERS[mode]()
        if not for_sim:
            _split_multi_waits(nc)
        _NC_CACHE[key] = nc
    return _NC_CACHE[key]


def _ones_blockdiag():
    ones = np.zeros((128, QPG), dtype=np.float32)
    for q in range(QPG):
        ones[q * TQ:(q + 1) * TQ, q] = 1.0
    return ones


def _to_fp8(x):
    import ml_dtypes
    return np.clip(x, -240.0, 240.0).astype(ml_dtypes.float8_e4m3fn)


def _make_in_maps(qs, ps, mode=MODE):
    qs = np.ascontiguousarray(np.asarray(qs), dtype=np.float32)
    ps = np.ascontiguousarray(np.asarray(ps), dtype=np.float32)
    assert qs.shape == (NQ, TQ, D) and ps.shape == (ND, TD, D)
    ones = _ones_blockdiag()

    in_maps = []
    if mode in ("fast", "pair"):
        cvt = (_to_fp8 if (mode == "fast" and FDAT == FP8)
               else (lambda x: x.astype(np.float16)))
        qsT = cvt(np.ascontiguousarray(
            qs.reshape(QROWS, D).T))                            # [128, 2048]
        pe = ps[:, 0::2, :]
        po = ps[:, 1::2, :]
        pplus = (pe + po) * 0.5                                 # [64,512,128]
        pminus = (pe - po) * 0.5
        ident = np.eye(128, dtype=np.float16)
        for k in range(N_CORES):
            sh = slice(k * DOCS_PER_CORE, (k + 1) * DOCS_PER_CORE)
            pP = cvt(np.ascontiguousarray(
                pplus[sh].reshape(DOCS_PER_CORE * NPAIR, D).T))  # [128, 4096]
            pM = cvt(np.ascontiguousarray(
                pminus[sh].reshape(DOCS_PER_CORE * NPAIR, D).T))
            in_maps.append({"qsT": qsT, "psP": pP, "psM": pM,
                            "ident": ident, "ones": ones})
    else:
        qsT = np.ascontiguousarray(qs.reshape(QROWS, D).T)      # [128, 2048]
        for k in range(N_CORES):
            shard = ps[k * DOCS_PER_CORE:(k + 1) * DOCS_PER_CORE]
            psTk = np.ascontiguousarray(
                shard.reshape(DOCS_PER_CORE * TD, D).T)
            in_maps.append({"qsT": qsT, "psT": psTk, "ones": ones})
    return in_maps


def _gather(results):
    return np.concatenate(
        [results[k]["out"] for k in range(N_CORES)], axis=1)


def kernel(qs, ps):
    nc = _get_nc()
    in_maps = _make_in_maps(qs, ps)
    res = bass_utils.run_bass_kernel_spmd(
        nc, in_maps, core_ids=list(range(N_CORES)))
    return _gather(res.results)


def kernel_timed(qs, ps, trace_cores=None):
    """Run with NTFF tracing; returns (scores, BassKernelResults)."""
    nc = _get_nc()
    in_maps = _make_in_maps(qs, ps)
    res = bass_utils.run_bass_kernel_spmd(
        nc, in_maps, core_ids=list(range(N_CORES)), trace=True,
        trace_cores=trace_cores)
    return _gather(res.results), res

